# revision 1
# baseline (speedup 1.0000x reference)
"""ChannelGate (topk_masking) Trainium2 Bass kernel.

Strategy: pure data parallel over batch (B=32 -> 4 samples per core x 8 cores).
Per core, per sample (x layout [C=512, HW=3136] as 4 c-tiles [128, 3136]):
  phase 1: stream x, compute channel stats (ACT copy+accum for sum, DVE
           reduce_max for max), pixel stats (PE f32r ones-matmul for sum,
           DVE tt-max combine + PE transpose + DVE psum reduce for max).
  phase 2: top-256 sorted extraction via DVE max8/match_replace on [8, 512]
           stat rows; tiny MLP on PE (interleave folded into host-split
           even/odd W1); 7x7 conv via DRAM padded buffer + im2col DMAs +
           PE f32r matmuls (BN folded into weights host-side).
  phase 3: re-stream x; gate = PE row broadcast + ACT sigmoid (per-partition
           channel scale) + one fused DVE/GPSIMD scalar_tensor_tensor
           out = (sig + 1) * x; DMA out.
"""
import os
import numpy as np
from contextlib import ExitStack

import concourse.bass as bass
import concourse.tile as tile
from concourse import bacc, mybir
from concourse import bass_utils

F32 = mybir.dt.float32
F32R = mybir.dt.float32r
F8 = mybir.dt.float8e4
BF16 = mybir.dt.bfloat16
AF = mybir.ActivationFunctionType
ALU = mybir.AluOpType
AX = mybir.AxisListType

B, C, H, W = 32, 512, 56, 56
HW = H * W            # 3136
S = 4                 # samples per core
NCORES = 8
G = 4                 # c-tiles of 128 per sample
RED = 32              # MLP hidden
NPIX_CH = 25          # ceil(3136/128) pixel chunks for transposes
CH512 = [(i * 512, min(512, HW - i * 512)) for i in range((HW + 511) // 512)]
PW = 62               # padded conv map width/height
NEG = -1.0e30


def r32(ap):
    return ap.bitcast(F32R)


def build_program():
    nc = bacc.Bacc("TRN2", target_bir_lowering=False, debug=False,
                   num_devices=NCORES)

    x_d = nc.dram_tensor("x", [S, C, HW], F32R, kind="ExternalInput")
    y_d = nc.dram_tensor("y", [S, C, HW], F32, kind="ExternalOutput")
    w1e_d = nc.dram_tensor("w1e", [64, 4 * RED], F32, kind="ExternalInput")
    w1o_d = nc.dram_tensor("w1o", [64, 4 * RED], F32, kind="ExternalInput")
    w2t_d = nc.dram_tensor("w2t", [RED, C], F32, kind="ExternalInput")
    b1_d = nc.dram_tensor("b1c", [RED, 1], F32, kind="ExternalInput")
    b2_d = nc.dram_tensor("b2c", [128, G], F32, kind="ExternalInput")
    wc_d = nc.dram_tensor("wc", [98, 1], F32, kind="ExternalInput")
    id_d = nc.dram_tensor("ident", [128, 128], F32, kind="ExternalInput")
    ssc_d = nc.dram_tensor("sortscale", [8, 1], F32, kind="ExternalInput")
    k2_d = nc.dram_tensor("k2c", [1, 1], F32, kind="ExternalInput")
    pad_d = nc.dram_tensor("pad0", [S * 2 * PW * PW], BF16, kind="ExternalInput")
    flat_d = nc.dram_tensor("flatscr", [S, NPIX_CH * 128], F32, kind="Internal")

    with tile.TileContext(nc) as tc:
        with ExitStack() as ctx:
            build_core(ctx, tc, x_d, y_d, w1e_d, w1o_d, w2t_d, b1_d, b2_d,
                       wc_d, id_d, ssc_d, k2_d, pad_d, flat_d)
    nc.compile()
    return nc


def build_core(ctx, tc, x_d, y_d, w1e_d, w1o_d, w2t_d, b1_d, b2_d, wc_d,
               id_d, ssc_d, k2_d, pad_d, flat_d):
    nc = tc.nc

    cpool = ctx.enter_context(tc.tile_pool(name="consts", bufs=1))
    xt_pool = ctx.enter_context(tc.tile_pool(name="xt", bufs=4))
    tmp_pool = ctx.enter_context(tc.tile_pool(name="tmp", bufs=2))
    scr_pool = ctx.enter_context(tc.tile_pool(name="scr", bufs=1))
    row_pool = ctx.enter_context(tc.tile_pool(name="rows", bufs=2))
    ss_pool = ctx.enter_context(tc.tile_pool(name="ss", bufs=3))
    ssl_pool = ctx.enter_context(tc.tile_pool(name="ssl", bufs=2))
    bc_pool = ctx.enter_context(tc.tile_pool(name="bc", bufs=1))
    sig_pool = ctx.enter_context(tc.tile_pool(name="sig", bufs=2))
    imt_pool = ctx.enter_context(tc.tile_pool(name="imt", bufs=1))

    ps_small = ctx.enter_context(tc.tile_pool(name="ps_small", bufs=2,
                                              space="PSUM"))
    ps_tr = ctx.enter_context(tc.tile_pool(name="ps_tr", bufs=2, space="PSUM"))
    ps_bc = ctx.enter_context(tc.tile_pool(name="ps_bc", bufs=2, space="PSUM"))

    # ---- constants / weights in SBUF ----
    ident = cpool.tile([128, 128], F32)
    nc.sync.dma_start(ident[:], id_d.ap())
    ones_col = cpool.tile([128, 1], F32)
    nc.vector.memset(ones_col[:], 1.0)
    ones_row = cpool.tile([1, 128], F32)
    nc.vector.memset(ones_row[:], 1.0)
    onesr_d = nc.dram_tensor("onesr", [128, 128], F32R, kind="ExternalInput")
    ones_r = cpool.tile([128, 128], F32R)
    nc.sync.dma_start(ones_r[:], onesr_d.ap())
    ident_bf = cpool.tile([128, 128], BF16)
    nc.vector.tensor_copy(ident_bf[:], ident[:])
    w1e = cpool.tile([64, 4 * RED], F32)
    nc.sync.dma_start(w1e[:], w1e_d.ap())
    w1o = cpool.tile([64, 4 * RED], F32)
    nc.sync.dma_start(w1o[:], w1o_d.ap())
    w2t = cpool.tile([RED, C], F32)
    nc.sync.dma_start(w2t[:], w2t_d.ap())
    b1 = cpool.tile([RED, 1], F32)
    nc.sync.dma_start(b1[:], b1_d.ap())
    b2 = cpool.tile([128, G], F32)
    nc.sync.dma_start(b2[:], b2_d.ap())
    wc = cpool.tile([98, 1], F32)
    nc.sync.dma_start(wc[:], wc_d.ap())
    wc_bf = cpool.tile([98, 1], BF16)
    nc.vector.tensor_copy(wc_bf[:], wc[:])
    sortscale = cpool.tile([8, 1], F32)
    nc.sync.dma_start(sortscale[:], ssc_d.ap())
    k2 = cpool.tile([1, 1], F32)
    nc.sync.dma_start(k2[:], k2_d.ap())

    sc_sum = [cpool.tile([128, 4], F32, tag=f"scs{g}", name=f"scs{g}") for g in range(G)]
    sc_max = [cpool.tile([128, 4], F32, tag=f"scm{g}", name=f"scm{g}") for g in range(G)]
    sc = [cpool.tile([128, 8], F32, tag=f"sc{g}", name=f"scq{g}") for g in range(G)]
    srt = cpool.tile([8, C], F32)                        # sort rows
    srtd = cpool.tile([8, 256], F32)                     # sorted top-256
    tq = [cpool.tile([64, 8], F32, tag=f"tq{q}", name=f"tq{q}") for q in range(4)]
    h_sb = cpool.tile([RED, S], F32)
    sqw = [cpool.tile([128, S], F32, tag=f"sqw{g}", name=f"sqw{g}") for g in range(G)]

    ssS = ss_pool.tile([S, HW], F32, tag="ssbig")        # pixel sums
    ssM = ss_pool.tile([S, HW], F32, tag="ssbig")        # pixel maxes

    # ================= PHASE 1: stats =================
    for s in range(S):
        xt = []
        for g in range(G):
            t = xt_pool.tile([128, HW], F32R, tag="t")
            nc.sync.dma_start(t[:], x_d.ap()[s, g * 128:(g + 1) * 128, :])
            xt.append(t)
            scr = scr_pool.tile([128, HW], F8)
            nc.scalar.activation(scr[:], t[:].bitcast(F32), AF.Copy,
                                 accum_out=sc_sum[g][:, s:s + 1])
            nc.vector.reduce_max(sc_max[g][:, s:s + 1], t[:].bitcast(F32),
                                 axis=AX.X)

        # pixel sums: ones.T @ x over all 4 c-tiles, f32r
        srow = row_pool.tile([1, HW], F32, tag="row")
        for (off, wdt) in CH512:
            ps = ps_bc.tile([1, 512], F32, tag='psb')
            for g in range(G):
                nc.tensor.matmul(ps[0:1, 0:wdt], ones_r[:, 0:1],
                                 xt[g][:, off:off + wdt],
                                 start=(g == 0), stop=(g == G - 1))
            nc.scalar.copy(srow[0:1, off:off + wdt], ps[0:1, 0:wdt])
        nc.sync.dma_start(ssS[s:s + 1, :], srow[:])

        # pixel maxes: combine 4 c-tiles (serial in-place chain, bf16 out)
        t1g = tmp_pool.tile([128, HW], BF16, tag="t1g")
        nc.vector.tensor_tensor(t1g[:], xt[0][:].bitcast(F32),
                                xt[1][:].bitcast(F32), op=ALU.max)
        mx = tmp_pool.tile([128, HW], BF16, tag="mx")
        nc.vector.tensor_tensor(mx[:], xt[2][:].bitcast(F32),
                                xt[3][:].bitcast(F32), op=ALU.max)
        nc.vector.tensor_tensor(mx[:], mx[:], t1g[:], op=ALU.max)
        # transpose 128-wide pixel chunks, reduce over c in psum
        ssl = ssl_pool.tile([128, NPIX_CH], F32)
        nc.vector.memset(ssl[:], 0.0)
        for j in range(NPIX_CH):
            wdt = min(128, HW - j * 128)
            pst = ps_tr.tile([128, 128], BF16, tag='pst')
            nc.tensor.transpose(pst[0:wdt, :], mx[:, j * 128:j * 128 + wdt],
                                ident_bf[:])
            nc.vector.reduce_max(ssl[0:wdt, j:j + 1], pst[0:wdt, :], axis=AX.X)
        # flatten [128, 25] -> DRAM pixel order via transpose + 2 DMAs
        psf = ps_tr.tile([NPIX_CH, 128], F32, tag='pst')
        nc.tensor.transpose(psf[:], ssl[:], ident[:])
        sslt = ssl_pool.tile([NPIX_CH, 128], F32)
        nc.scalar.copy(sslt[:], psf[:])
        nc.sync.dma_start(flat_d.ap()[s, :].rearrange("(p f) -> p f",
                                                      p=NPIX_CH), sslt[:])
        nc.sync.dma_start(ssM[s:s + 1, :],
                          flat_d.ap()[s, 0:HW].rearrange("(p f) -> p f", p=1))

    # ================= PHASE 2: topk sort + MLP =================
    for g in range(G):
        nc.sync.dma_start(sc[g][:, 0:4], sc_sum[g][:])
        nc.sync.dma_start(sc[g][:, 4:8], sc_max[g][:])
        pst = ps_small.tile([8, 128], F32, tag='pss')
        nc.tensor.transpose(pst[:], sc[g][:], ident[:])
        nc.scalar.activation(srt[:, g * 128:(g + 1) * 128], pst[:], AF.Copy,
                             scale=sortscale[:])
    for it in range(32):
        m8 = srtd[:, 8 * it:8 * it + 8]
        nc.vector.max(out=m8, in_=srt[:])
        nc.vector.match_replace(out=srt[:], in_to_replace=m8,
                                in_values=srt[:], imm_value=NEG)
    # transpose sorted rows into [64, 8] chunks (cols 0-3 t1, 4-7 t2)
    for q in range(4):
        pst = ps_small.tile([64, 8], F32, tag='pss')
        nc.tensor.transpose(pst[:], srtd[:, 64 * q:64 * q + 64],
                            ident[0:8, 0:8])
        nc.scalar.copy(tq[q][:], pst[:])
    # h = relu(W1e @ t1 + W1o @ t2 + b1)
    psh = ps_small.tile([RED, S], F32, tag='pss')
    for q in range(4):
        c0 = q * RED
        lhs_e = w1e[:, c0:c0 + RED]
        lhs_o = w1o[:, c0:c0 + RED]
        nc.tensor.matmul(psh[:], lhs_e, tq[q][:, 0:4],
                         start=(q == 0), stop=False)
        nc.tensor.matmul(psh[:], lhs_o, tq[q][:, 4:8],
                         start=False, stop=(q == 3))
    nc.scalar.activation(h_sb[:], psh[:], AF.Relu, bias=b1[:])
    # mlp_out per c-tile; squeeze_weight = relu(mlp_out + b2 + sigmoid(sc1*sc2))
    for g in range(G):
        psm = ps_small.tile([128, S], F32, tag='pss')
        nc.tensor.matmul(psm[:], w2t[:, g * 128:(g + 1) * 128], h_sb[:],
                         start=True, stop=True)
        prod = cpool.tile([128, S], F32, tag=f"prod{g}")
        nc.vector.tensor_tensor(prod[:], sc[g][:, 0:4], sc[g][:, 4:8],
                                op=ALU.mult)
        sigp = cpool.tile([128, S], F32, tag=f"sigp{g}")
        nc.scalar.activation(sigp[:], prod[:], AF.Sigmoid, scale=1.0 / HW)
        nc.vector.tensor_tensor(sigp[:], sigp[:], psm[:], op=ALU.add)
        nc.scalar.activation(sqw[g][:], sigp[:], AF.Relu, bias=b2[:, g:g + 1])

    # ================= SPATIAL PATH =================
    prodS = ss_pool.tile([S, HW], F32, tag="ssbig")
    nc.vector.tensor_tensor(prodS[:], ssS[:], ssM[:], op=ALU.mult)
    nc.scalar.activation(prodS[:], prodS[:], AF.Sigmoid, scale=1.0 / C)
    # conv: padded interiors -> im2col -> PE matmuls -> bn bias
    for s in range(S):
        for ci, src2 in ((0, ssS), (1, ssM)):
            base = ((s * 2 + ci) * PW + 3) * PW + 3
            dst = bass.AP(pad_d, base, [[PW, H], [1, W]])
            nc.gpsimd.dma_start(dst,
                                src2[s:s + 1, :].rearrange("p (h w) -> p h w",
                                                           h=H))
    cb = ss_pool.tile([S, HW], F32, tag="ssbig")
    for s in range(S):
        imt = imt_pool.tile([98, HW], BF16)
        for ci in range(2):
            for kh in range(7):
                base = ((s * 2 + ci) * PW + kh) * PW
                src = bass.AP(pad_d, base, [[1, 7], [PW, H], [1, W]])
                p0 = ci * 49 + kh * 7
                nc.sync.dma_start(imt[p0:p0 + 7, :], src)
        crow = row_pool.tile([1, HW], F32, tag="row")
        for (off, wdt) in CH512:
            psc = ps_bc.tile([1, 512], F32, tag='psb')
            nc.tensor.matmul(psc[0:1, 0:wdt], wc_bf[:],
                             imt[:, off:off + wdt], start=True, stop=True)
            nc.scalar.activation(crow[0:1, off:off + wdt], psc[0:1, 0:wdt],
                                 AF.Identity, bias=k2[0:1, 0:1])
        nc.sync.dma_start(cb[s:s + 1, :], crow[:])
    spw = ss_pool.tile([S, HW], F32, tag="ssbig")
    nc.vector.tensor_tensor(spw[:], cb[:], prodS[:], op=ALU.add)

    # ================= PHASE 3: gate =================
    for s in range(S):
        spr = row_pool.tile([1, HW], F32R, tag="rowr", bufs=1)
        nc.gpsimd.dma_start(spr[:], spw[s:s + 1, :])
        bcS = bc_pool.tile([128, HW], F32)
        for (off, wdt) in CH512:
            psb = ps_bc.tile([128, 512], F32, tag='psb')
            nc.tensor.matmul(psb[:, 0:wdt], ones_r[0:1, :],
                             spr[0:1, off:off + wdt],
                             start=True, stop=True)
            nc.scalar.copy(bcS[:, off:off + wdt], psb[:, 0:wdt])
        for g in range(G):
            xg = xt_pool.tile([128, HW], F32R, tag="t")
            nc.sync.dma_start(xg[:], x_d.ap()[s, g * 128:(g + 1) * 128, :])
            sg = sig_pool.tile([128, HW], F32)
            nc.scalar.activation(sg[:], bcS[:], AF.Sigmoid,
                                 scale=sqw[g][:, s:s + 1])
            nc.vector.scalar_tensor_tensor(sg[:], in0=sg[:], scalar=1.0,
                                           in1=xg[:].bitcast(F32),
                                           op0=ALU.add, op1=ALU.mult)
            nc.sync.dma_start(y_d.ap()[s, g * 128:(g + 1) * 128, :], sg[:])


_NC_CACHE = {}


def _get_program():
    if "nc" not in _NC_CACHE:
        _NC_CACHE["nc"] = build_program()
    return _NC_CACHE["nc"]


def _host_params(w1, b1, w2, b2, conv_w, bn_gamma, bn_beta, bn_mean, bn_var):
    w1 = np.asarray(w1, np.float32)
    w2 = np.asarray(w2, np.float32)
    b1 = np.asarray(b1, np.float32)
    b2 = np.asarray(b2, np.float32)
    conv_w = np.asarray(conv_w, np.float32)

    w1e = np.ascontiguousarray(
        w1[:, 0::2].T.reshape(4, 64, RED).transpose(1, 0, 2).reshape(64, 4 * RED))
    w1o = np.ascontiguousarray(
        w1[:, 1::2].T.reshape(4, 64, RED).transpose(1, 0, 2).reshape(64, 4 * RED))
    w2t = np.ascontiguousarray(w2.T)                    # [32, 512]
    b1c = b1.reshape(RED, 1).copy()
    b2c = np.ascontiguousarray(b2.reshape(G, 128).T)    # [128, G]

    bn_scale = float(bn_gamma[0]) / np.sqrt(float(bn_var[0]) + 1e-5)
    k2 = float(bn_beta[0]) - float(bn_mean[0]) * bn_scale
    wcf = conv_w[0].astype(np.float64) * bn_scale       # [2, 7, 7]
    wcf = wcf.copy()
    wcf[0] /= C                                         # mean channel fold
    wc = wcf.reshape(98, 1).astype(np.float32)

    sortscale = np.concatenate([np.full(4, 1.0 / HW, np.float32),
                                np.ones(4, np.float32)]).reshape(8, 1)
    ident = np.eye(128, dtype=np.float32)
    k2c = np.array([[k2]], np.float32)
    onesr = np.ones((128, 128), np.float32)
    import ml_dtypes
    pad0 = np.zeros(S * 2 * PW * PW, ml_dtypes.bfloat16)
    return dict(w1e=w1e, w1o=w1o, w2t=w2t, b1c=b1c, b2c=b2c, wc=wc,
                ident=ident, sortscale=sortscale, k2c=k2c, pad0=pad0,
                onesr=onesr)


def kernel(x, w1, b1, w2, b2, conv_w, bn_gamma, bn_beta, bn_mean, bn_var):
    x = np.asarray(x, np.float32)
    params = _host_params(w1, b1, w2, b2, conv_w,
                          bn_gamma, bn_beta, bn_mean, bn_var)
    nc = _get_program()

    xr = x.reshape(B, C, HW)
    in_maps = []
    for k in range(NCORES):
        m = {"x": np.ascontiguousarray(xr[k * S:(k + 1) * S])}
        m.update(params)
        in_maps.append(m)

    res = bass_utils.run_bass_kernel_spmd(nc, in_maps,
                                          core_ids=list(range(NCORES)))
    out = np.concatenate([res.results[k]["y"] for k in range(NCORES)], axis=0)
    return out.reshape(B, C, H, W).astype(np.float32)



# revision 5
# speedup vs baseline: 1.2084x; 1.2084x over previous
"""ChannelGate (topk_masking) Trainium2 Bass kernel.

Strategy: pure data parallel over batch (B=32 -> 4 samples per core x 8 cores).
fp16 end-to-end: host casts x to fp16 (halves HBM reads), y is written fp16
(halves writes).  x c-tiles 0-2 stay resident in SBUF between the stats pass
and the gating pass; tile 3 is re-streamed.

Per core, per sample (x layout [C=512, HW=3136] as 4 c-tiles [128, 3136]):
  phase 1: ch-sum via DVE scalar_tensor_tensor accum (4x fp16 rate), ch-max
           via fp16 fold trees + small reduce, pixel sum via PE ones-matmul
           on the c-tile sum tree, pixel max via GPSIMD partition_all_reduce
           on the c-tile max tree.  7x7 conv (BN folded) computed here too:
           DRAM padded buffer -> im2col DMA -> PE matmuls (bias via ones row).
  phase 2: top-256 sorted extraction: 16 x max8/match_replace on a [32, 128]
           quarter layout, then exact bitonic merges on [8, 512] rows; tiny
           MLP on PE (interleave folded into host-split even/odd W1).
  phase 3: psum chunk = ones x (sigmoid-spatial row) + ones x (conv row);
           ACT sigmoid with per-channel scale; DVE stt out = (sig + 1) * x
           in place over the resident x tile; DMA out fp16.
"""
import numpy as np
from contextlib import ExitStack

import concourse.bass as bass
import concourse.tile as tile
from concourse import bacc, mybir, bass_isa
from concourse import bass_utils

F32 = mybir.dt.float32
F16 = mybir.dt.float16
AF = mybir.ActivationFunctionType
ALU = mybir.AluOpType
AX = mybir.AxisListType

B, C, H, W = 32, 512, 56, 56
HW = H * W            # 3136
S = 4                 # samples per core
NCORES = 8
G = 4                 # c-tiles of 128 per sample
RED = 32              # MLP hidden
PW = 62               # padded conv map width/height
CH512 = [(i * 512, min(512, HW - i * 512)) for i in range((HW + 511) // 512)]
NEG = -1.0e30
HALF = HW // 2        # 1568


def build_program():
    nc = bacc.Bacc("TRN2", target_bir_lowering=False, debug=False,
                   num_devices=NCORES)

    x_d = nc.dram_tensor("x", [S, C, HW], F16, kind="ExternalInput")
    y_d = nc.dram_tensor("y", [S, C, HW], F16, kind="ExternalOutput")
    w1e_d = nc.dram_tensor("w1e", [64, 4 * RED], F16, kind="ExternalInput")
    w1o_d = nc.dram_tensor("w1o", [64, 4 * RED], F16, kind="ExternalInput")
    w2t_d = nc.dram_tensor("w2t", [RED, C], F16, kind="ExternalInput")
    b1_d = nc.dram_tensor("b1c", [RED, 1], F32, kind="ExternalInput")
    b2_d = nc.dram_tensor("b2c", [128, G], F32, kind="ExternalInput")
    wcr_d = nc.dram_tensor("wcrep", [99, 128], F16, kind="ExternalInput")
    id_d = nc.dram_tensor("ident", [128, 128], F32, kind="ExternalInput")
    ssc_d = nc.dram_tensor("sortscale", [8, 1], F32, kind="ExternalInput")
    pad_d = nc.dram_tensor("pad0", [S * 2 * PW * PW], F16, kind="ExternalInput")

    with tile.TileContext(nc) as tc:
        with ExitStack() as ctx:
            build_core(ctx, tc, x_d, y_d, w1e_d, w1o_d, w2t_d, b1_d, b2_d,
                       wcr_d, id_d, ssc_d, pad_d)
    nc.compile()
    return nc


def build_core(ctx, tc, x_d, y_d, w1e_d, w1o_d, w2t_d, b1_d, b2_d, wcr_d,
               id_d, ssc_d, pad_d):
    nc = tc.nc

    cpool = ctx.enter_context(tc.tile_pool(name="consts", bufs=1))
    big = ctx.enter_context(tc.tile_pool(name="big", bufs=5))
    t3p = ctx.enter_context(tc.tile_pool(name="t3p", bufs=2))
    rowp = ctx.enter_context(tc.tile_pool(name="rows", bufs=1))
    imt_pool = ctx.enter_context(tc.tile_pool(name="imt", bufs=1))

    ps_pix = ctx.enter_context(tc.tile_pool(name="ps_pix", bufs=2,
                                            space="PSUM"))
    ps_sm = ctx.enter_context(tc.tile_pool(name="ps_sm", bufs=2, space="PSUM"))
    ps_bc = ctx.enter_context(tc.tile_pool(name="ps_bc", bufs=2, space="PSUM"))

    # ---- constants / weights in SBUF ----
    ident = cpool.tile([128, 128], F32)
    nc.sync.dma_start(ident[:], id_d.ap())
    ones1 = cpool.tile([1, 128], F16)
    nc.vector.memset(ones1[:], 1.0)
    onescol = cpool.tile([128, 1], F16)
    nc.vector.memset(onescol[:], 1.0)
    w1e = cpool.tile([64, 4 * RED], F16)
    nc.sync.dma_start(w1e[:], w1e_d.ap())
    w1o = cpool.tile([64, 4 * RED], F16)
    nc.sync.dma_start(w1o[:], w1o_d.ap())
    w2t = cpool.tile([RED, C], F16)
    nc.sync.dma_start(w2t[:], w2t_d.ap())
    b1 = cpool.tile([RED, 1], F32)
    nc.sync.dma_start(b1[:], b1_d.ap())
    b2 = cpool.tile([128, G], F32)
    nc.sync.dma_start(b2[:], b2_d.ap())
    wcrep = cpool.tile([99, 128], F16)
    nc.sync.dma_start(wcrep[:], wcr_d.ap())
    ssc = cpool.tile([8, 1], F32)
    nc.sync.dma_start(ssc[:], ssc_d.ap())

    # resident x tiles (c-tiles 0..2)
    xt = [[cpool.tile([128, HW], F16, tag=f"x{s}{g}", name=f"x{s}{g}")
           for g in range(3)] for s in range(S)]
    # per-(g) channel stats: cols 0-3 sums, 4-7 maxes
    sc8 = [cpool.tile([128, 8], F32, tag=f"sc8{g}", name=f"sc8{g}")
           for g in range(G)]
    # per-sample rows: cols [0:HW] sig-spatial (starts as pix sums),
    # cols [HW:2HW] conv result
    srow = [rowp.tile([1, 2 * HW], F16, tag=f"srow{s}", name=f"srow{s}")
            for s in range(S)]
    fold = cpool.tile([128, HALF], F16)       # ch-max fold scratch
    imt = imt_pool.tile([99, HW], F16)        # im2col + ones row
    nc.vector.memset(imt[96:99, :], 1.0)

    # sort tiles
    srtF = cpool.tile([8, C], F32)
    srt32 = cpool.tile([32, 128], F32)
    sorted32 = cpool.tile([32, 128], F32)
    q512 = cpool.tile([8, C], F32)
    mg1 = cpool.tile([8, C], F32)
    mg2 = cpool.tile([8, C], F32)
    tq = [cpool.tile([64, 8], F16, tag=f"tq{q}", name=f"tq{q}")
          for q in range(4)]
    h_sb = cpool.tile([RED, S], F16)
    sqw = [cpool.tile([128, S], F32, tag=f"sqw{g}", name=f"sqw{g}")
           for g in range(G)]

    def stt(out, in0, in1, op1, accum=None):
        nc.vector.scalar_tensor_tensor(out, in0=in0, scalar=1.0, in1=in1,
                                       op0=ALU.mult, op1=op1,
                                       accum_out=accum)

    # ================= PHASE 1 =================
    for s in range(S):
        ts = []
        for g in range(G):
            if g < 3:
                t = xt[s][g]
            else:
                t = t3p.tile([128, HW], F16, tag="t3")
            nc.sync.dma_start(t[:], x_d.ap()[s, g * 128:(g + 1) * 128, :])
            ts.append(t)

        A = big.tile([128, HW], F16, tag="big")
        Bt = big.tile([128, HW], F16, tag="big")
        Ct = big.tile([128, HW], F16, tag="big")

        # channel stats per c-tile
        for g in range(G):
            # ch-sum: identity stt with f32 accumulator (garbage out -> Ct)
            stt(Ct[:], ts[g][:], ts[g][:], ALU.max,
                accum=sc8[g][:, s:s + 1])
            # ch-max: fp16 fold tree + small reduce
            stt(fold[:], ts[g][:, 0:HALF], ts[g][:, HALF:HW], ALU.max)
            stt(fold[:, 0:784], fold[:, 0:784], fold[:, 784:1568], ALU.max)
            stt(fold[:, 0:392], fold[:, 0:392], fold[:, 392:784], ALU.max)
            nc.vector.reduce_max(sc8[g][:, 4 + s:5 + s], fold[:, 0:392],
                                 axis=AX.X)

        # pixel trees
        stt(A[:], ts[0][:], ts[1][:], ALU.add)
        stt(Bt[:], ts[2][:], ts[3][:], ALU.add)
        stt(A[:], A[:], Bt[:], ALU.add)          # A = channel-sum tree
        stt(Ct[:], ts[0][:], ts[1][:], ALU.max)
        stt(Bt[:], ts[2][:], ts[3][:], ALU.max)
        stt(Ct[:], Ct[:], Bt[:], ALU.max)        # Ct = channel-max tree

        # pixel sums: ones.T @ A -> psum -> srow cols [0:HW]
        for (off, wdt) in CH512:
            ps = ps_pix.tile([1, 512], F32, tag="pix")
            nc.tensor.matmul(ps[0:1, 0:wdt], onescol[:, 0:1],
                             A[:, off:off + wdt], start=True, stop=True)
            nc.scalar.copy(srow[s][0:1, off:off + wdt], ps[0:1, 0:wdt])

        # pixel maxes: cross-partition reduce on GPSIMD, result -> A
        nc.gpsimd.partition_all_reduce(A[:], Ct[:], channels=128,
                                       reduce_op=bass_isa.ReduceOp.max)

        # conv pad interiors (pre-zeroed DRAM buffer)
        base0 = ((s * 2 + 0) * PW + 3) * PW + 3
        base1 = ((s * 2 + 1) * PW + 3) * PW + 3
        nc.sync.dma_start(
            bass.AP(pad_d, base0, [[PW, H], [1, W]]),
            srow[s][0:1, 0:HW].rearrange("p (h w) -> p h w", h=H))
        nc.sync.dma_start(
            bass.AP(pad_d, base1, [[PW, H], [1, W]]),
            A[0:1, 0:HW].rearrange("p (h w) -> p h w", h=H))

        # sig-spatial row: sigmoid(ss1 * ss2 / C); /C and the ss1 mean fold
        # are host-folded into the sigmoid scale (1/C) and conv weights
        stt(srow[s][0:1, 0:HW], srow[s][0:1, 0:HW], A[0:1, 0:HW], ALU.mult)
        nc.scalar.activation(srow[s][0:1, 0:HW], srow[s][0:1, 0:HW],
                             AF.Sigmoid, scale=1.0 / C)

        # im2col loads (one DMA per (ci, kh))
        for ci in range(2):
            for kh in range(7):
                base = ((s * 2 + ci) * PW + kh) * PW
                src = bass.AP(pad_d, base, [[1, 7], [PW, H], [1, W]])
                p0 = ci * 49 + kh * 7
                nc.sync.dma_start(imt[p0:p0 + 7, :], src)
        # conv matmuls (bias via ones row folded into wcrep row 98)
        for (off, wdt) in CH512:
            psc = ps_pix.tile([1, 512], F32, tag="pix")
            nc.tensor.matmul(psc[0:1, 0:wdt], wcrep[:, 0:1],
                             imt[:, off:off + wdt], start=True, stop=True)
            nc.scalar.copy(srow[s][0:1, HW + off:HW + off + wdt],
                           psc[0:1, 0:wdt])

    # ================= PHASE 2: sort + MLP =================
    for g in range(G):
        pst = ps_sm.tile([8, 128], F32, tag="pst")
        nc.tensor.transpose(pst[:], sc8[g][:], ident[:])
        nc.scalar.activation(srtF[:, g * 128:(g + 1) * 128], pst[:], AF.Copy,
                             scale=ssc[:])
    # rearrange [8, 512] -> [32, 128] (quarter g of row r at partition 8g+r)
    for g in range(G):
        nc.sync.dma_start(srt32[8 * g:8 * g + 8, :],
                          srtF[0:8, g * 128:(g + 1) * 128])
    # full sort of each 128-quarter: 16 x (max8 + match_replace)
    for it in range(16):
        m8 = sorted32[:, 8 * it:8 * it + 8]
        nc.vector.max(out=m8, in_=srt32[:])
        nc.vector.match_replace(out=srt32[:], in_to_replace=m8,
                                in_values=srt32[:], imm_value=NEG)
    # back to [8, 512] rows
    for g in range(G):
        nc.sync.dma_start(q512[0:8, g * 128:(g + 1) * 128],
                          sorted32[8 * g:8 * g + 8, :])
    # merge quarters pairwise (128+128 -> 256 sorted desc), both pairs at once
    nc.vector.tensor_tensor(mg1[:, 0:128], q512[:, 0:128],
                            q512[:, 255:127:-1], op=ALU.max)
    nc.vector.tensor_tensor(mg1[:, 128:256], q512[:, 0:128],
                            q512[:, 255:127:-1], op=ALU.min)
    nc.vector.tensor_tensor(mg1[:, 256:384], q512[:, 256:384],
                            q512[:, 511:383:-1], op=ALU.max)
    nc.vector.tensor_tensor(mg1[:, 384:512], q512[:, 256:384],
                            q512[:, 511:383:-1], op=ALU.min)
    cur, nxt = mg1, mg2
    for d in (64, 32, 16, 8, 4, 2, 1):
        cv = cur[:].rearrange("p (x two d) -> p x two d", two=2, d=d)
        nv = nxt[:].rearrange("p (x two d) -> p x two d", two=2, d=d)
        nc.vector.tensor_tensor(nv[:, :, 0, :], cv[:, :, 0, :],
                                cv[:, :, 1, :], op=ALU.max)
        nc.vector.tensor_tensor(nv[:, :, 1, :], cv[:, :, 0, :],
                                cv[:, :, 1, :], op=ALU.min)
        cur, nxt = nxt, cur
    # final merge: top-256 of the two 256-lists, sorted desc
    nc.vector.tensor_tensor(nxt[:, 0:256], cur[:, 0:256],
                            cur[:, 511:255:-1], op=ALU.max)
    cur, nxt = nxt, cur
    for d in (128, 64, 32, 16, 8, 4, 2, 1):
        cv = cur[:, 0:256].rearrange("p (x two d) -> p x two d", two=2, d=d)
        nv = nxt[:, 0:256].rearrange("p (x two d) -> p x two d", two=2, d=d)
        nc.vector.tensor_tensor(nv[:, :, 0, :], cv[:, :, 0, :],
                                cv[:, :, 1, :], op=ALU.max)
        nc.vector.tensor_tensor(nv[:, :, 1, :], cv[:, :, 0, :],
                                cv[:, :, 1, :], op=ALU.min)
        cur, nxt = nxt, cur
    # cur[:, 0:256] = top-256 sorted desc; rows 0-3 = t1 samples, 4-7 = t2
    for q in range(4):
        pstq = ps_sm.tile([64, 8], F32, tag="pst")
        nc.tensor.transpose(pstq[:], cur[0:8, 64 * q:64 * q + 64],
                            ident[0:8, 0:8])
        nc.scalar.copy(tq[q][:], pstq[:])
    # h = relu(W1e @ t1 + W1o @ t2 + b1)
    psh = ps_sm.tile([RED, S], F32, tag="pst")
    for q in range(4):
        c0 = q * RED
        nc.tensor.matmul(psh[:], w1e[:, c0:c0 + RED], tq[q][:, 0:4],
                         start=(q == 0), stop=False)
        nc.tensor.matmul(psh[:], w1o[:, c0:c0 + RED], tq[q][:, 4:8],
                         start=False, stop=(q == 3))
    nc.scalar.activation(h_sb[:], psh[:], AF.Relu, bias=b1[:])
    # squeeze_weight = relu(mlp_out + b2 + sigmoid(sc1*sc2))
    for g in range(G):
        psm = ps_sm.tile([128, S], F32, tag="pst")
        nc.tensor.matmul(psm[:], w2t[:, g * 128:(g + 1) * 128], h_sb[:],
                         start=True, stop=True)
        prod = cpool.tile([128, S], F32, tag=f"prod{g}", name=f"prod{g}")
        nc.vector.tensor_tensor(prod[:], sc8[g][:, 0:4], sc8[g][:, 4:8],
                                op=ALU.mult)
        sigp = cpool.tile([128, S], F32, tag=f"sigp{g}", name=f"sigp{g}")
        nc.scalar.activation(sigp[:], prod[:], AF.Sigmoid, scale=1.0 / HW)
        nc.vector.tensor_tensor(sigp[:], sigp[:], psm[:], op=ALU.add)
        nc.scalar.activation(sqw[g][:], sigp[:], AF.Relu, bias=b2[:, g:g + 1])

    # ================= PHASE 3: gate =================
    for s in range(S):
        t3 = t3p.tile([128, HW], F16, tag="t3")
        nc.sync.dma_start(t3[:], x_d.ap()[s, 3 * 128:4 * 128, :])
        tl = [xt[s][0], xt[s][1], xt[s][2], t3]
        sgt = [big.tile([128, HW], F16, tag="big", name=f"sgt{s}{g}")
               for g in range(G)]
        for (off, wdt) in CH512:
            psb = ps_bc.tile([128, 512], F32, tag="bc")
            nc.tensor.matmul(psb[:, 0:wdt], ones1[0:1, :],
                             srow[s][0:1, off:off + wdt],
                             start=True, stop=False)
            nc.tensor.matmul(psb[:, 0:wdt], ones1[0:1, :],
                             srow[s][0:1, HW + off:HW + off + wdt],
                             start=False, stop=True)
            for g in range(G):
                nc.scalar.activation(sgt[g][:, off:off + wdt], psb[:, 0:wdt],
                                     AF.Sigmoid, scale=sqw[g][:, s:s + 1])
        for g in range(G):
            nc.vector.scalar_tensor_tensor(tl[g][:], in0=sgt[g][:],
                                           scalar=1.0, in1=tl[g][:],
                                           op0=ALU.add, op1=ALU.mult)
            nc.sync.dma_start(y_d.ap()[s, g * 128:(g + 1) * 128, :], tl[g][:])


_NC_CACHE = {}


def _get_program():
    if "nc" not in _NC_CACHE:
        _NC_CACHE["nc"] = build_program()
    return _NC_CACHE["nc"]


def _host_params(w1, b1, w2, b2, conv_w, bn_gamma, bn_beta, bn_mean, bn_var):
    w1 = np.asarray(w1, np.float32)
    w2 = np.asarray(w2, np.float32)
    b1 = np.asarray(b1, np.float32)
    b2 = np.asarray(b2, np.float32)
    conv_w = np.asarray(conv_w, np.float32)

    w1e = np.ascontiguousarray(
        w1[:, 0::2].T.reshape(4, 64, RED).transpose(1, 0, 2)
        .reshape(64, 4 * RED)).astype(np.float16)
    w1o = np.ascontiguousarray(
        w1[:, 1::2].T.reshape(4, 64, RED).transpose(1, 0, 2)
        .reshape(64, 4 * RED)).astype(np.float16)
    w2t = np.ascontiguousarray(w2.T).astype(np.float16)    # [32, 512]
    b1c = b1.reshape(RED, 1).copy()
    b2c = np.ascontiguousarray(b2.reshape(G, 128).T)       # [128, G]

    bn_scale = float(bn_gamma[0]) / np.sqrt(float(bn_var[0]) + 1e-5)
    k2 = float(bn_beta[0]) - float(bn_mean[0]) * bn_scale
    wcf = conv_w[0].astype(np.float64) * bn_scale          # [2, 7, 7]
    wcf = wcf.copy()
    wcf[0] /= C                                            # mean channel fold
    wc99 = np.concatenate([wcf.reshape(98, 1),
                           np.array([[k2]])], axis=0)      # [99, 1]
    wcrep = np.ascontiguousarray(
        np.broadcast_to(wc99, (99, 128))).astype(np.float16)

    sortscale = np.concatenate([np.full(4, 1.0 / HW, np.float32),
                                np.ones(4, np.float32)]).reshape(8, 1)
    ident = np.eye(128, dtype=np.float32)
    pad0 = np.zeros(S * 2 * PW * PW, np.float16)
    return dict(w1e=w1e, w1o=w1o, w2t=w2t, b1c=b1c, b2c=b2c, wcrep=wcrep,
                ident=ident, sortscale=sortscale, pad0=pad0)


def kernel(x, w1, b1, w2, b2, conv_w, bn_gamma, bn_beta, bn_mean, bn_var):
    x = np.asarray(x, np.float32)
    params = _host_params(w1, b1, w2, b2, conv_w,
                          bn_gamma, bn_beta, bn_mean, bn_var)
    nc = _get_program()

    xr = x.reshape(B, C, HW).astype(np.float16)
    in_maps = []
    for k in range(NCORES):
        m = {"x": np.ascontiguousarray(xr[k * S:(k + 1) * S])}
        m.update(params)
        in_maps.append(m)

    res = bass_utils.run_bass_kernel_spmd(nc, in_maps,
                                          core_ids=list(range(NCORES)))
    out = np.concatenate([np.asarray(res.results[k]["y"], np.float32)
                          for k in range(NCORES)], axis=0)
    return out.reshape(B, C, H, W)


# revision 8
# speedup vs baseline: 1.8538x; 1.5341x over previous
"""ChannelGate (topk_masking) Trainium2 Bass kernel.

Strategy: pure data parallel over batch (B=32 -> 4 samples per core x 8 cores).
fp16 end-to-end: host casts x to fp16 (halves HBM reads), y is written fp16
(halves writes).  x c-tiles 0-2 stay resident in SBUF between the stats pass
and the gating pass; tile 3 is re-streamed.

Op selection is driven by measured TRN2 DVE rates: tensor_copy/tensor_scalar
~0.37 ns/elem, tensor_tensor ~0.6, reduce ~1.2, scalar_tensor_tensor ~1.3
(no fast mode) -- so everything elementwise uses TT/TS, never STT.

Per core, per sample (x layout [C=512, HW=3136] as 4 c-tiles [128, 3136]):
  phase 1: ch-sum via ACT copy+accum (fp16 in), ch-max via TT fold tree +
           small reduce, pixel sum via 4-way accumulating PE ones-matmuls,
           pixel max via TT max tree + GPSIMD partition_all_reduce.
  phase 2: top-256 sorted extraction: 16 x max8/match_replace on a [32, 128]
           quarter layout (fp16), then exact bitonic merges on [8, 512] rows;
           tiny MLP on PE (interleave folded into host-split even/odd W1).
  phase 3: psum chunk = wcrep.T @ im2col  +  ones x sig-spatial row (conv,
           BN and bias folded host-side; bcast replicated to 128 rows); ACT
           sigmoid with per-channel scale; DVE TS (+1) and TT (*x) in place
           over the resident x tile; DMA out fp16.
"""
import numpy as np
from contextlib import ExitStack

import concourse.bass as bass
import concourse.tile as tile
from concourse import bacc, mybir, bass_isa
from concourse import bass_utils

F32 = mybir.dt.float32
F16 = mybir.dt.float16
F8 = mybir.dt.float8e4
AF = mybir.ActivationFunctionType
ALU = mybir.AluOpType
AX = mybir.AxisListType

B, C, H, W = 32, 512, 56, 56
HW = H * W            # 3136
S = 4                 # samples per core
NCORES = 8
G = 4                 # c-tiles of 128 per sample
RED = 32              # MLP hidden
PW = 62               # padded conv map width/height
CH512 = [(i * 512, min(512, HW - i * 512)) for i in range((HW + 511) // 512)]
NEG = -60000.0
HALF = HW // 2        # 1568


def build_program():
    nc = bacc.Bacc("TRN2", target_bir_lowering=False, debug=False,
                   num_devices=NCORES)

    x_d = nc.dram_tensor("x", [S, C, HW], F16, kind="ExternalInput")
    y_d = nc.dram_tensor("y", [S, C, HW], F16, kind="ExternalOutput")
    w1e_d = nc.dram_tensor("w1e", [64, 4 * RED], F16, kind="ExternalInput")
    w1o_d = nc.dram_tensor("w1o", [64, 4 * RED], F16, kind="ExternalInput")
    w2t_d = nc.dram_tensor("w2t", [RED, C], F16, kind="ExternalInput")
    b1_d = nc.dram_tensor("b1c", [RED, 1], F32, kind="ExternalInput")
    b2_d = nc.dram_tensor("b2c", [128, G], F32, kind="ExternalInput")
    wcr_d = nc.dram_tensor("wcrep", [99, 128], F16, kind="ExternalInput")
    id_d = nc.dram_tensor("ident", [128, 128], F32, kind="ExternalInput")
    ssc_d = nc.dram_tensor("sortscale", [8, 1], F32, kind="ExternalInput")
    pad_d = nc.dram_tensor("pad0", [S * 2 * PW * PW], F16, kind="ExternalInput")
    or_d = nc.dram_tensor("onesrow", [1, HW], F16, kind="ExternalInput")

    with tile.TileContext(nc) as tc:
        with ExitStack() as ctx:
            build_core(ctx, tc, x_d, y_d, w1e_d, w1o_d, w2t_d, b1_d, b2_d,
                       wcr_d, id_d, ssc_d, pad_d, or_d)
    nc.compile()
    return nc


def build_core(ctx, tc, x_d, y_d, w1e_d, w1o_d, w2t_d, b1_d, b2_d, wcr_d,
               id_d, ssc_d, pad_d, or_d):
    nc = tc.nc

    cpool = ctx.enter_context(tc.tile_pool(name="consts", bufs=1))
    big = ctx.enter_context(tc.tile_pool(name="big", bufs=5))
    t3p = ctx.enter_context(tc.tile_pool(name="t3p", bufs=2))
    rowp = ctx.enter_context(tc.tile_pool(name="rows", bufs=1))

    ps_pix = ctx.enter_context(tc.tile_pool(name="ps_pix", bufs=2,
                                            space="PSUM"))
    ps_sm = ctx.enter_context(tc.tile_pool(name="ps_sm", bufs=2, space="PSUM"))
    ps_bc = ctx.enter_context(tc.tile_pool(name="ps_bc", bufs=3, space="PSUM"))

    # ---- constants / weights in SBUF ----
    ident = cpool.tile([128, 128], F32)
    nc.sync.dma_start(ident[:], id_d.ap())
    ones1 = cpool.tile([1, 128], F16)
    nc.vector.memset(ones1[:], 1.0)
    onescol = cpool.tile([128, 1], F16)
    nc.vector.memset(onescol[:], 1.0)
    w1e = cpool.tile([64, 4 * RED], F16)
    nc.sync.dma_start(w1e[:], w1e_d.ap())
    w1o = cpool.tile([64, 4 * RED], F16)
    nc.sync.dma_start(w1o[:], w1o_d.ap())
    w2t = cpool.tile([RED, C], F16)
    nc.sync.dma_start(w2t[:], w2t_d.ap())
    b1 = cpool.tile([RED, 1], F32)
    nc.sync.dma_start(b1[:], b1_d.ap())
    b2 = cpool.tile([128, G], F32)
    nc.sync.dma_start(b2[:], b2_d.ap())
    wcrep = cpool.tile([99, 128], F16)
    nc.sync.dma_start(wcrep[:], wcr_d.ap())
    ssc = cpool.tile([8, 1], F32)
    nc.sync.dma_start(ssc[:], ssc_d.ap())
    ident16 = cpool.tile([8, 8], F16)
    nc.vector.tensor_copy(ident16[:], ident[0:8, 0:8])

    # resident x tiles (c-tiles 0..2)
    xt = [[cpool.tile([128, HW], F16, tag=f"x{s}{g}", name=f"x{s}{g}")
           for g in range(3)] for s in range(S)]
    # per-(g) channel stats: cols 0-3 sums, 4-7 maxes
    sc8 = [cpool.tile([128, 8], F32, tag=f"sc8{g}", name=f"sc8{g}")
           for g in range(G)]
    # per-sample spatial row: pixel sums -> sigmoid(ss1*ss2/C)
    srow = [rowp.tile([1, HW], F16, tag=f"srow{s}", name=f"srow{s}")
            for s in range(S)]
    # per-sample im2col tiles (persist until the phase-3 conv matmuls)
    imts = [cpool.tile([99, HW], F16, tag=f"imt{s}", name=f"imt{s}")
            for s in range(S)]
    for s in range(S):
        nc.sync.dma_start(imts[s][98:99, :], or_d.ap())
    fold = cpool.tile([128, HALF], F16)       # ch-max fold scratch
    scrap = cpool.tile([128, HW], F8)         # ch-sum ACT copy target

    # sort tiles (fp16 values)
    srtF = cpool.tile([8, C], F16)
    srt32 = cpool.tile([32, 128], F16)
    sorted32 = cpool.tile([32, 128], F16)
    q512 = cpool.tile([8, C], F16)
    mg1 = cpool.tile([8, C], F16)
    mg2 = cpool.tile([8, C], F16)
    tq = [cpool.tile([64, 8], F16, tag=f"tq{q}", name=f"tq{q}")
          for q in range(4)]
    h_sb = cpool.tile([RED, S], F16)
    sqw = [cpool.tile([128, S], F32, tag=f"sqw{g}", name=f"sqw{g}")
           for g in range(G)]

    # ================= PHASE 1 =================
    for s in range(S):
        ts = []
        for g in range(G):
            if g < 3:
                t = xt[s][g]
            else:
                t = t3p.tile([128, HW], F16, tag="t3")
            nc.sync.dma_start(t[:], x_d.ap()[s, g * 128:(g + 1) * 128, :])
            ts.append(t)

        Bt = big.tile([128, HW], F16, tag="big")
        Ct = big.tile([128, HW], F16, tag="big")

        # channel stats per c-tile
        for g in range(G):
            # ch-sum: ACT copy + f32 accumulator
            nc.scalar.activation(scrap[:], ts[g][:], AF.Copy,
                                 accum_out=sc8[g][:, s:s + 1])
            # ch-max: fp16 TT fold tree + small reduce
            nc.vector.tensor_tensor(fold[:], ts[g][:, 0:HALF],
                                    ts[g][:, HALF:HW], op=ALU.max)
            nc.vector.tensor_tensor(fold[:, 0:784], fold[:, 0:784],
                                    fold[:, 784:1568], op=ALU.max)
            nc.vector.reduce_max(sc8[g][:, 4 + s:5 + s], fold[:, 0:784],
                                 axis=AX.X)

        # pixel sums: 4-way accumulating ones-matmuls per 512-chunk
        for (off, wdt) in CH512:
            ps = ps_pix.tile([1, 512], F32, tag="pix")
            for g in range(G):
                nc.tensor.matmul(ps[0:1, 0:wdt], onescol[:, 0:1],
                                 ts[g][:, off:off + wdt],
                                 start=(g == 0), stop=(g == 3))
            nc.scalar.copy(srow[s][0:1, off:off + wdt], ps[0:1, 0:wdt])

        # pixel maxes: TT max tree + cross-partition reduce on GPSIMD -> Bt
        nc.vector.tensor_tensor(Ct[:], ts[0][:], ts[1][:], op=ALU.max)
        nc.vector.tensor_tensor(Bt[:], ts[2][:], ts[3][:], op=ALU.max)
        nc.vector.tensor_tensor(Ct[:], Ct[:], Bt[:], op=ALU.max)
        nc.gpsimd.partition_all_reduce(Bt[:], Ct[:], channels=128,
                                       reduce_op=bass_isa.ReduceOp.max)

        # conv pad interiors (pre-zeroed DRAM buffer)
        base0 = ((s * 2 + 0) * PW + 3) * PW + 3
        base1 = ((s * 2 + 1) * PW + 3) * PW + 3
        nc.sync.dma_start(
            bass.AP(pad_d, base0, [[PW, H], [1, W]]),
            srow[s][0:1, 0:HW].rearrange("p (h w) -> p h w", h=H))
        nc.sync.dma_start(
            bass.AP(pad_d, base1, [[PW, H], [1, W]]),
            Bt[0:1, 0:HW].rearrange("p (h w) -> p h w", h=H))

        # sig-spatial row: sigmoid(ss1 * ss2 / C); ss1 mean fold is in the
        # conv weights / sigmoid scale host-side
        nc.vector.tensor_tensor(srow[s][0:1, 0:HW], srow[s][0:1, 0:HW],
                                Bt[0:1, 0:HW], op=ALU.mult)
        nc.scalar.activation(srow[s][0:1, 0:HW], srow[s][0:1, 0:HW],
                             AF.Sigmoid, scale=1.0 / C)

        # im2col loads (one DMA per (ci, kh))
        for ci in range(2):
            for kh in range(7):
                base = ((s * 2 + ci) * PW + kh) * PW
                src = bass.AP(pad_d, base, [[1, 7], [PW, H], [1, W]])
                p0 = ci * 49 + kh * 7
                nc.sync.dma_start(imts[s][p0:p0 + 7, :], src)

    # ================= PHASE 2: sort + MLP =================
    for g in range(G):
        pst = ps_sm.tile([8, 128], F32, tag="pst")
        nc.tensor.transpose(pst[:], sc8[g][:], ident[:])
        nc.scalar.activation(srtF[:, g * 128:(g + 1) * 128], pst[:], AF.Copy,
                             scale=ssc[:])
    # rearrange [8, 512] -> [32, 128] (quarter g of row r at partition 8g+r)
    for g in range(G):
        nc.sync.dma_start(srt32[8 * g:8 * g + 8, :],
                          srtF[0:8, g * 128:(g + 1) * 128])
    # full sort of each 128-quarter: 16 x (max8 + match_replace)
    for it in range(16):
        m8 = sorted32[:, 8 * it:8 * it + 8]
        nc.vector.max(out=m8, in_=srt32[:])
        nc.vector.match_replace(out=srt32[:], in_to_replace=m8,
                                in_values=srt32[:], imm_value=NEG)
    # back to [8, 512] rows
    for g in range(G):
        nc.sync.dma_start(q512[0:8, g * 128:(g + 1) * 128],
                          sorted32[8 * g:8 * g + 8, :])
    # merge quarters pairwise (128+128 -> 256 sorted desc), both pairs at once
    nc.vector.tensor_tensor(mg1[:, 0:128], q512[:, 0:128],
                            q512[:, 255:127:-1], op=ALU.max)
    nc.vector.tensor_tensor(mg1[:, 128:256], q512[:, 0:128],
                            q512[:, 255:127:-1], op=ALU.min)
    nc.vector.tensor_tensor(mg1[:, 256:384], q512[:, 256:384],
                            q512[:, 511:383:-1], op=ALU.max)
    nc.vector.tensor_tensor(mg1[:, 384:512], q512[:, 256:384],
                            q512[:, 511:383:-1], op=ALU.min)
    cur, nxt = mg1, mg2
    for d in (64, 32, 16, 8, 4, 2, 1):
        cv = cur[:].rearrange("p (x two d) -> p x two d", two=2, d=d)
        nv = nxt[:].rearrange("p (x two d) -> p x two d", two=2, d=d)
        nc.vector.tensor_tensor(nv[:, :, 0, :], cv[:, :, 0, :],
                                cv[:, :, 1, :], op=ALU.max)
        nc.vector.tensor_tensor(nv[:, :, 1, :], cv[:, :, 0, :],
                                cv[:, :, 1, :], op=ALU.min)
        cur, nxt = nxt, cur
    # final merge: top-256 of the two 256-lists, sorted desc
    nc.vector.tensor_tensor(nxt[:, 0:256], cur[:, 0:256],
                            cur[:, 511:255:-1], op=ALU.max)
    cur, nxt = nxt, cur
    for d in (128, 64, 32, 16, 8, 4, 2, 1):
        cv = cur[:, 0:256].rearrange("p (x two d) -> p x two d", two=2, d=d)
        nv = nxt[:, 0:256].rearrange("p (x two d) -> p x two d", two=2, d=d)
        nc.vector.tensor_tensor(nv[:, :, 0, :], cv[:, :, 0, :],
                                cv[:, :, 1, :], op=ALU.max)
        nc.vector.tensor_tensor(nv[:, :, 1, :], cv[:, :, 0, :],
                                cv[:, :, 1, :], op=ALU.min)
        cur, nxt = nxt, cur
    # cur[:, 0:256] = top-256 sorted desc; rows 0-3 = t1 samples, 4-7 = t2
    for q in range(4):
        pstq = ps_sm.tile([64, 8], F16, tag="pst")
        nc.tensor.transpose(pstq[:], cur[0:8, 64 * q:64 * q + 64],
                            ident16[:])
        nc.scalar.copy(tq[q][:], pstq[:])
    # h = relu(W1e @ t1 + W1o @ t2 + b1)
    psh = ps_sm.tile([RED, S], F32, tag="pst")
    for q in range(4):
        c0 = q * RED
        nc.tensor.matmul(psh[:], w1e[:, c0:c0 + RED], tq[q][:, 0:4],
                         start=(q == 0), stop=False)
        nc.tensor.matmul(psh[:], w1o[:, c0:c0 + RED], tq[q][:, 4:8],
                         start=False, stop=(q == 3))
    nc.scalar.activation(h_sb[:], psh[:], AF.Relu, bias=b1[:])
    # squeeze_weight = relu(mlp_out + b2 + sigmoid(sc1*sc2))
    for g in range(G):
        psm = ps_sm.tile([128, S], F32, tag="pst")
        nc.tensor.matmul(psm[:], w2t[:, g * 128:(g + 1) * 128], h_sb[:],
                         start=True, stop=True)
        prod = cpool.tile([128, S], F32, tag=f"prod{g}", name=f"prod{g}")
        nc.vector.tensor_tensor(prod[:], sc8[g][:, 0:4], sc8[g][:, 4:8],
                                op=ALU.mult)
        sigp = cpool.tile([128, S], F32, tag=f"sigp{g}", name=f"sigp{g}")
        nc.scalar.activation(sigp[:], prod[:], AF.Sigmoid, scale=1.0 / HW)
        nc.vector.tensor_tensor(sigp[:], sigp[:], psm[:], op=ALU.add)
        nc.scalar.activation(sqw[g][:], sigp[:], AF.Relu, bias=b2[:, g:g + 1])

    # ================= PHASE 3: gate =================
    for s in range(S):
        t3 = t3p.tile([128, HW], F16, tag="t3")
        nc.sync.dma_start(t3[:], x_d.ap()[s, 3 * 128:4 * 128, :])
        tl = [xt[s][0], xt[s][1], xt[s][2], t3]
        sgt = [big.tile([128, HW], F16, tag="big", name=f"sgt{s}{g}")
               for g in range(G)]
        for (off, wdt) in CH512:
            psb = ps_bc.tile([128, 512], F32, tag="bc")
            nc.tensor.matmul(psb[:, 0:wdt], wcrep[:],
                             imts[s][:, off:off + wdt],
                             start=True, stop=False, skip_group_check=True)
            nc.tensor.matmul(psb[:, 0:wdt], ones1[0:1, :],
                             srow[s][0:1, off:off + wdt],
                             start=False, stop=True, skip_group_check=True)
            for g in range(G):
                nc.scalar.activation(sgt[g][:, off:off + wdt], psb[:, 0:wdt],
                                     AF.Sigmoid, scale=sqw[g][:, s:s + 1])
        for g in range(G):
            nc.vector.tensor_scalar(out=sgt[g][:], in0=sgt[g][:],
                                    scalar1=1.0, scalar2=None, op0=ALU.add)
            nc.vector.tensor_tensor(tl[g][:], tl[g][:], sgt[g][:],
                                    op=ALU.mult)
            nc.sync.dma_start(y_d.ap()[s, g * 128:(g + 1) * 128, :], tl[g][:])


_NC_CACHE = {}


def _get_program():
    if "nc" not in _NC_CACHE:
        _NC_CACHE["nc"] = build_program()
    return _NC_CACHE["nc"]


def _host_params(w1, b1, w2, b2, conv_w, bn_gamma, bn_beta, bn_mean, bn_var):
    w1 = np.asarray(w1, np.float32)
    w2 = np.asarray(w2, np.float32)
    b1 = np.asarray(b1, np.float32)
    b2 = np.asarray(b2, np.float32)
    conv_w = np.asarray(conv_w, np.float32)

    w1e = np.ascontiguousarray(
        w1[:, 0::2].T.reshape(4, 64, RED).transpose(1, 0, 2)
        .reshape(64, 4 * RED)).astype(np.float16)
    w1o = np.ascontiguousarray(
        w1[:, 1::2].T.reshape(4, 64, RED).transpose(1, 0, 2)
        .reshape(64, 4 * RED)).astype(np.float16)
    w2t = np.ascontiguousarray(w2.T).astype(np.float16)    # [32, 512]
    b1c = b1.reshape(RED, 1).copy()
    b2c = np.ascontiguousarray(b2.reshape(G, 128).T)       # [128, G]

    bn_scale = float(bn_gamma[0]) / np.sqrt(float(bn_var[0]) + 1e-5)
    k2 = float(bn_beta[0]) - float(bn_mean[0]) * bn_scale
    wcf = conv_w[0].astype(np.float64) * bn_scale          # [2, 7, 7]
    wcf = wcf.copy()
    wcf[0] /= C                                            # mean channel fold
    wc99 = np.concatenate([wcf.reshape(98, 1),
                           np.array([[k2]])], axis=0)      # [99, 1]
    wcrep = np.ascontiguousarray(
        np.broadcast_to(wc99, (99, 128))).astype(np.float16)

    sortscale = np.concatenate([np.full(4, 1.0 / HW, np.float32),
                                np.ones(4, np.float32)]).reshape(8, 1)
    ident = np.eye(128, dtype=np.float32)
    pad0 = np.zeros(S * 2 * PW * PW, np.float16)
    onesrow = np.ones((1, HW), np.float16)
    return dict(w1e=w1e, w1o=w1o, w2t=w2t, b1c=b1c, b2c=b2c, wcrep=wcrep,
                ident=ident, sortscale=sortscale, pad0=pad0, onesrow=onesrow)


def kernel(x, w1, b1, w2, b2, conv_w, bn_gamma, bn_beta, bn_mean, bn_var):
    x = np.asarray(x, np.float32)
    params = _host_params(w1, b1, w2, b2, conv_w,
                          bn_gamma, bn_beta, bn_mean, bn_var)
    nc = _get_program()

    xr = x.reshape(B, C, HW).astype(np.float16)
    in_maps = []
    for k in range(NCORES):
        m = {"x": np.ascontiguousarray(xr[k * S:(k + 1) * S])}
        m.update(params)
        in_maps.append(m)

    res = bass_utils.run_bass_kernel_spmd(nc, in_maps,
                                          core_ids=list(range(NCORES)))
    out = np.concatenate([np.asarray(res.results[k]["y"], np.float32)
                          for k in range(NCORES)], axis=0)
    return out.reshape(B, C, H, W)


# revision 13
# speedup vs baseline: 1.9343x; 1.0434x over previous
"""ChannelGate (topk_masking) Trainium2 Bass kernel.

Strategy: pure data parallel over batch (B=32 -> 4 samples per core x 8 cores).
fp16 end-to-end: host casts x to fp16 (halves HBM reads), y is written fp16
(halves writes).  x c-tiles 0-2 stay resident in SBUF between the stats pass
and the gating pass; tile 3 is re-streamed.

Op selection is driven by measured TRN2 DVE rates: tensor_copy/tensor_scalar
~0.37 ns/elem, tensor_tensor ~0.6, reduce ~1.2, scalar_tensor_tensor ~1.3
(no fast mode) -- so everything elementwise uses TT/TS, never STT.

Two-sample groups pipeline through three phases so the top-k sort and MLP of
group 0 overlap phase 1 of group 1, and gating of group 0 overlaps the tail:
  phase 1 (per sample): ch-sum via ACT copy+accum, ch-max via TT fold tree +
           small reduce, pixel sum via 4-way accumulating PE ones-matmuls,
           pixel max via TT max tree + GPSIMD partition_all_reduce; conv
           im2col staged via a pre-zeroed DRAM pad buffer.
  phase 2 (per group): top-256 sorted: 16 x max8/match_replace on a [16, 128]
           quarter layout (fp16), exact bitonic merges on [4, 512] rows;
           tiny MLP on PE (interleave folded into host-split even/odd W1).
  phase 3 (per sample): psum = wcrep.T @ im2col + ones x sig-spatial row
           (conv, BN, bias folded host-side; bcast replicated to 128 rows);
           ACT sigmoid with per-channel scale; DVE TS (+1) and TT (*x) in
           place over the resident x tile; DMA out fp16.
"""
import numpy as np
from contextlib import ExitStack

import concourse.bass as bass
import concourse.tile as tile
from concourse import bacc, mybir, bass_isa
from concourse import bass_utils

F32 = mybir.dt.float32
F16 = mybir.dt.float16
F8 = mybir.dt.float8e4
AF = mybir.ActivationFunctionType
ALU = mybir.AluOpType
AX = mybir.AxisListType

B, C, H, W = 32, 512, 56, 56
HW = H * W            # 3136
S = 4                 # samples per core
NCORES = 8
G = 4                 # c-tiles of 128 per sample
RED = 32              # MLP hidden
PW = 62               # padded conv map width/height
CH512 = [(i * 512, min(512, HW - i * 512)) for i in range((HW + 511) // 512)]
CH1K = [(0, 1024), (1024, 1024), (2048, 1024), (3072, 64)]
NEG = -60000.0
HALF = HW // 2        # 1568


def build_program():
    nc = bacc.Bacc("TRN2", target_bir_lowering=False, debug=False,
                   num_devices=NCORES)

    x_d = nc.dram_tensor("x", [S, C, HW], F16, kind="ExternalInput")
    y_d = nc.dram_tensor("y", [S, C, HW], F16, kind="ExternalOutput")
    w1e_d = nc.dram_tensor("w1e", [64, 4 * RED], F16, kind="ExternalInput")
    w1o_d = nc.dram_tensor("w1o", [64, 4 * RED], F16, kind="ExternalInput")
    w2t_d = nc.dram_tensor("w2t", [RED, C], F16, kind="ExternalInput")
    b1_d = nc.dram_tensor("b1c", [RED, 1], F32, kind="ExternalInput")
    b2_d = nc.dram_tensor("b2c", [128, G], F32, kind="ExternalInput")
    wcr_d = nc.dram_tensor("wcrep", [99, 128], F16, kind="ExternalInput")
    id_d = nc.dram_tensor("ident", [128, 128], F32, kind="ExternalInput")
    ssc_d = nc.dram_tensor("sortscale", [4, 1], F32, kind="ExternalInput")
    pad_d = nc.dram_tensor("pad0", [S * 2 * PW * PW], F16, kind="ExternalInput")
    or_d = nc.dram_tensor("onesrow", [1, HW], F16, kind="ExternalInput")

    with tile.TileContext(nc) as tc:
        with ExitStack() as ctx:
            Kern(ctx, tc, x_d, y_d, w1e_d, w1o_d, w2t_d, b1_d, b2_d,
                 wcr_d, id_d, ssc_d, pad_d, or_d).build()
    nc.compile()
    return nc


class Kern:
    def __init__(self, ctx, tc, x_d, y_d, w1e_d, w1o_d, w2t_d, b1_d, b2_d,
                 wcr_d, id_d, ssc_d, pad_d, or_d):
        self.ctx, self.tc, self.nc = ctx, tc, tc.nc
        self.x_d, self.y_d = x_d, y_d
        self.w1e_d, self.w1o_d, self.w2t_d = w1e_d, w1o_d, w2t_d
        self.b1_d, self.b2_d, self.wcr_d = b1_d, b2_d, wcr_d
        self.id_d, self.ssc_d, self.pad_d, self.or_d = (id_d, ssc_d, pad_d,
                                                        or_d)

    def build(self):
        ctx, tc, nc = self.ctx, self.tc, self.nc
        cpool = ctx.enter_context(tc.tile_pool(name="consts", bufs=1))
        self.big = ctx.enter_context(tc.tile_pool(name="big", bufs=5))
        self.t3p = ctx.enter_context(tc.tile_pool(name="t3p", bufs=2))
        rowp = ctx.enter_context(tc.tile_pool(name="rows", bufs=1))
        self.ps_pix = ctx.enter_context(
            tc.tile_pool(name="ps_pix", bufs=2, space="PSUM"))
        self.ps_sm = ctx.enter_context(
            tc.tile_pool(name="ps_sm", bufs=2, space="PSUM"))
        self.ps_bc = ctx.enter_context(
            tc.tile_pool(name="ps_bc", bufs=2, space="PSUM"))

        # resident x tiles + first sample loads go out before the consts
        self.xt = [[cpool.tile([128, HW], F16, tag=f"x{s}{g}", name=f"x{s}{g}")
                    for g in range(3)] for s in range(S)]
        self.t3_1 = {}
        self.load_x(0)

        ident = cpool.tile([128, 128], F32)
        nc.sync.dma_start(ident[:], self.id_d.ap())
        self.ident = ident
        self.ones1 = cpool.tile([1, 128], F16)
        nc.vector.memset(self.ones1[:], 1.0)
        self.onescol = cpool.tile([128, 1], F16)
        nc.vector.memset(self.onescol[:], 1.0)
        self.w1e = cpool.tile([64, 4 * RED], F16)
        nc.sync.dma_start(self.w1e[:], self.w1e_d.ap())
        self.w1o = cpool.tile([64, 4 * RED], F16)
        nc.sync.dma_start(self.w1o[:], self.w1o_d.ap())
        self.w2t = cpool.tile([RED, C], F16)
        nc.sync.dma_start(self.w2t[:], self.w2t_d.ap())
        self.b1 = cpool.tile([RED, 1], F32)
        nc.sync.dma_start(self.b1[:], self.b1_d.ap())
        self.b2 = cpool.tile([128, G], F32)
        nc.sync.dma_start(self.b2[:], self.b2_d.ap())
        self.wcrep = cpool.tile([99, 128], F16)
        nc.sync.dma_start(self.wcrep[:], self.wcr_d.ap())
        self.ssc = cpool.tile([4, 1], F32)
        nc.sync.dma_start(self.ssc[:], self.ssc_d.ap())
        self.ident16 = cpool.tile([8, 8], F16)
        nc.vector.tensor_copy(self.ident16[:], ident[0:8, 0:8])

        self.sc8 = [cpool.tile([128, 8], F32, tag=f"sc8{g}", name=f"sc8{g}")
                    for g in range(G)]
        self.srow = [rowp.tile([1, HW], F16, tag=f"srow{s}", name=f"srow{s}")
                     for s in range(S)]
        self.imts = [cpool.tile([99, HW], F16, tag=f"imt{s}", name=f"imt{s}")
                     for s in range(S)]
        for s in range(S):
            nc.gpsimd.dma_start(self.imts[s][98:99, :], self.or_d.ap())
        self.fold = cpool.tile([128, HALF], F16)
        self.scrap = cpool.tile([128, HW], F8)

        # per-group sort tiles
        self.srtF = [cpool.tile([4, C], F16, tag=f"srtF{k}", name=f"srtF{k}")
                     for k in range(2)]
        self.srt16 = [cpool.tile([16, 128], F16, tag=f"sr16{k}",
                                 name=f"sr16{k}") for k in range(2)]
        self.sorted16 = [cpool.tile([16, 128], F16, tag=f"so16{k}",
                                    name=f"so16{k}") for k in range(2)]
        self.q512 = [cpool.tile([4, C], F16, tag=f"q512{k}", name=f"q512{k}")
                     for k in range(2)]
        self.mg1 = [cpool.tile([4, C], F16, tag=f"mg1{k}", name=f"mg1{k}")
                    for k in range(2)]
        self.mg2 = [cpool.tile([4, C], F16, tag=f"mg2{k}", name=f"mg2{k}")
                    for k in range(2)]
        self.tq = [cpool.tile([64, 4], F16, tag=f"tq{q}", name=f"tq{q}")
                   for q in range(4)]
        self.h_sb = [cpool.tile([RED, 2], F16, tag=f"hsb{k}", name=f"hsb{k}")
                     for k in range(2)]
        self.sqw = [cpool.tile([128, S], F32, tag=f"sqw{g}", name=f"sqw{g}")
                    for g in range(G)]
        self.prod = [cpool.tile([128, 2], F32, tag=f"prod{g}",
                                name=f"prod{g}") for g in range(G)]
        self.sigp = [cpool.tile([128, 2], F32, tag=f"sigp{g}",
                                name=f"sigp{g}") for g in range(G)]

        # ---- pipeline ----
        self.phase1(0)
        self.load_x(1)
        self.phase1(1)
        self.load_x(2)
        self.sort_mlp(0)
        self.phase1(2)
        self.load_x(3)
        self.phase3(0)
        self.phase1(3)
        self.phase3(1)
        self.sort_mlp(1)
        self.phase3(2)
        self.phase3(3)

    def load_x(self, s):
        nc = self.nc
        for g in range(G):
            if g < 3:
                t = self.xt[s][g]
            else:
                t = self.t3p.tile([128, HW], F16, tag="t3", name=f"t3a{s}")
                self.t3_1[s] = t
            nc.sync.dma_start(t[:], self.x_d.ap()[s, g * 128:(g + 1) * 128, :])

    def phase1(self, s):
        nc = self.nc
        ts = [self.xt[s][0], self.xt[s][1], self.xt[s][2], self.t3_1[s]]
        sc8, srow, fold = self.sc8, self.srow[s], self.fold

        Bt = self.big.tile([128, HW], F16, tag="big", name=f"Bt{s}")
        Ct = self.big.tile([128, HW], F16, tag="big", name=f"Ct{s}")

        cs = 4 * (s // 2) + (s % 2)      # per-group col block: s0,s1,m0,m1
        cm = cs + 2
        for g in range(G):
            nc.scalar.activation(self.scrap[:], ts[g][:], AF.Copy,
                                 accum_out=sc8[g][:, cs:cs + 1])
            nc.vector.tensor_tensor(fold[:], ts[g][:, 0:HALF],
                                    ts[g][:, HALF:HW], op=ALU.max)
            nc.vector.tensor_tensor(fold[:, 0:784], fold[:, 0:784],
                                    fold[:, 784:1568], op=ALU.max)
            nc.vector.reduce_max(sc8[g][:, cm:cm + 1], fold[:, 0:784],
                                 axis=AX.X)

        for (off, wdt) in CH512:
            ps = self.ps_pix.tile([1, 512], F32, tag="pix")
            for g in range(G):
                nc.tensor.matmul(ps[0:1, 0:wdt], self.onescol[:, 0:1],
                                 ts[g][:, off:off + wdt],
                                 start=(g == 0), stop=(g == 3))
            nc.scalar.copy(srow[0:1, off:off + wdt], ps[0:1, 0:wdt])

        nc.vector.tensor_tensor(Ct[:], ts[0][:], ts[1][:], op=ALU.max)
        nc.vector.tensor_tensor(Bt[:], ts[2][:], ts[3][:], op=ALU.max)
        nc.vector.tensor_tensor(Ct[:], Ct[:], Bt[:], op=ALU.max)
        nc.gpsimd.partition_all_reduce(Bt[:], Ct[:], channels=128,
                                       reduce_op=bass_isa.ReduceOp.max)

        base0 = ((s * 2 + 0) * PW + 3) * PW + 3
        base1 = ((s * 2 + 1) * PW + 3) * PW + 3
        nc.sync.dma_start(
            bass.AP(self.pad_d, base0, [[PW, H], [1, W]]),
            srow[0:1, 0:HW].rearrange("p (h w) -> p h w", h=H))
        nc.gpsimd.dma_start(
            bass.AP(self.pad_d, base1, [[PW, H], [1, W]]),
            Bt[0:1, 0:HW].rearrange("p (h w) -> p h w", h=H))

        nc.vector.tensor_tensor(srow[0:1, 0:HW], srow[0:1, 0:HW],
                                Bt[0:1, 0:HW], op=ALU.mult)
        nc.scalar.activation(srow[0:1, 0:HW], srow[0:1, 0:HW],
                             AF.Sigmoid, scale=1.0 / C)

        # im2col loads: ci 0 on HWDGE (sync), ci 1 on SWDGE (gpsimd)
        for ci in range(2):
            eng = nc.sync if ci == 0 else nc.gpsimd
            for kh in range(7):
                base = ((s * 2 + ci) * PW + kh) * PW
                src = bass.AP(self.pad_d, base, [[1, 7], [PW, H], [1, W]])
                p0 = ci * 49 + kh * 7
                eng.dma_start(self.imts[s][p0:p0 + 7, :], src)

    def sort_mlp(self, k):
        nc = self.nc
        s0 = 2 * k
        srtF, srt16, sorted16 = self.srtF[k], self.srt16[k], self.sorted16[k]
        q512, mg1, mg2 = self.q512[k], self.mg1[k], self.mg2[k]

        # stats -> [4, 512] rows (r = pool*2 + si), sums scaled by 1/HW
        for g in range(G):
            pst = self.ps_sm.tile([4, 128], F32, tag="pst")
            nc.tensor.transpose(pst[:], self.sc8[g][:, 4 * k:4 * k + 4],
                                self.ident[:])
            nc.scalar.activation(srtF[:, g * 128:(g + 1) * 128], pst[:],
                                 AF.Copy, scale=self.ssc[:])
        for g in range(G):
            nc.sync.dma_start(srt16[4 * g:4 * g + 4, :],
                              srtF[0:4, g * 128:(g + 1) * 128])
        for it in range(16):
            m8 = sorted16[:, 8 * it:8 * it + 8]
            nc.vector.max(out=m8, in_=srt16[:])
            nc.vector.match_replace(out=srt16[:], in_to_replace=m8,
                                    in_values=srt16[:], imm_value=NEG)
        for g in range(G):
            nc.sync.dma_start(q512[0:4, g * 128:(g + 1) * 128],
                              sorted16[4 * g:4 * g + 4, :])
        # merge quarters pairwise (both pairs at once)
        nc.vector.tensor_tensor(mg1[:, 0:128], q512[:, 0:128],
                                q512[:, 255:127:-1], op=ALU.max)
        nc.vector.tensor_tensor(mg1[:, 128:256], q512[:, 0:128],
                                q512[:, 255:127:-1], op=ALU.min)
        nc.vector.tensor_tensor(mg1[:, 256:384], q512[:, 256:384],
                                q512[:, 511:383:-1], op=ALU.max)
        nc.vector.tensor_tensor(mg1[:, 384:512], q512[:, 256:384],
                                q512[:, 511:383:-1], op=ALU.min)
        cur, nxt = mg1, mg2
        for d in (64, 32, 16, 8, 4, 2, 1):
            cv = cur[:].rearrange("p (x two d) -> p x two d", two=2, d=d)
            nv = nxt[:].rearrange("p (x two d) -> p x two d", two=2, d=d)
            nc.vector.tensor_tensor(nv[:, :, 0, :], cv[:, :, 0, :],
                                    cv[:, :, 1, :], op=ALU.max)
            nc.vector.tensor_tensor(nv[:, :, 1, :], cv[:, :, 0, :],
                                    cv[:, :, 1, :], op=ALU.min)
            cur, nxt = nxt, cur
        nc.vector.tensor_tensor(nxt[:, 0:256], cur[:, 0:256],
                                cur[:, 511:255:-1], op=ALU.max)
        cur, nxt = nxt, cur
        for d in (128, 64, 32, 16, 8, 4, 2, 1):
            cv = cur[:, 0:256].rearrange("p (x two d) -> p x two d",
                                         two=2, d=d)
            nv = nxt[:, 0:256].rearrange("p (x two d) -> p x two d",
                                         two=2, d=d)
            nc.vector.tensor_tensor(nv[:, :, 0, :], cv[:, :, 0, :],
                                    cv[:, :, 1, :], op=ALU.max)
            nc.vector.tensor_tensor(nv[:, :, 1, :], cv[:, :, 0, :],
                                    cv[:, :, 1, :], op=ALU.min)
            cur, nxt = nxt, cur
        # cur[:, 0:256] = top-256 sorted desc; rows 0-1 = t1, 2-3 = t2
        for q in range(4):
            pstq = self.ps_sm.tile([64, 4], F16, tag="pst")
            nc.tensor.transpose(pstq[:], cur[0:4, 64 * q:64 * q + 64],
                                self.ident16[0:4, 0:4])
            nc.scalar.copy(self.tq[q][:], pstq[:])
        psh = self.ps_sm.tile([RED, 2], F32, tag="pst")
        for q in range(4):
            c0 = q * RED
            nc.tensor.matmul(psh[:], self.w1e[:, c0:c0 + RED],
                             self.tq[q][:, 0:2], start=(q == 0), stop=False)
            nc.tensor.matmul(psh[:], self.w1o[:, c0:c0 + RED],
                             self.tq[q][:, 2:4], start=False, stop=(q == 3))
        nc.scalar.activation(self.h_sb[k][:], psh[:], AF.Relu, bias=self.b1[:])
        for g in range(G):
            psm = self.ps_sm.tile([128, 2], F32, tag="pst")
            nc.tensor.matmul(psm[:], self.w2t[:, g * 128:(g + 1) * 128],
                             self.h_sb[k][:], start=True, stop=True)
            nc.vector.tensor_tensor(self.prod[g][:],
                                    self.sc8[g][:, 4 * k:4 * k + 2],
                                    self.sc8[g][:, 4 * k + 2:4 * k + 4],
                                    op=ALU.mult)
            nc.scalar.activation(self.sigp[g][:], self.prod[g][:], AF.Sigmoid,
                                 scale=1.0 / HW)
            nc.vector.tensor_tensor(self.sigp[g][:], self.sigp[g][:], psm[:],
                                    op=ALU.add)
            nc.scalar.activation(self.sqw[g][:, s0:s0 + 2], self.sigp[g][:],
                                 AF.Relu, bias=self.b2[:, g:g + 1])

    def phase3(self, s):
        nc = self.nc
        t3 = self.t3p.tile([128, HW], F16, tag="t3", name=f"t3b{s}")
        nc.sync.dma_start(t3[:], self.x_d.ap()[s, 3 * 128:4 * 128, :])
        tl = [self.xt[s][0], self.xt[s][1], self.xt[s][2], t3]
        sgt = [self.big.tile([128, HW], F16, tag="big", name=f"sgt{s}{g}")
               for g in range(G)]
        for (off, wdt) in CH1K:
            psb = self.ps_bc.tile([128, 1024], F32, tag="bc")
            for h0 in range(0, wdt, 512):
                hw_ = min(512, wdt - h0)
                nc.tensor.matmul(psb[:, h0:h0 + hw_], self.wcrep[:],
                                 self.imts[s][:, off + h0:off + h0 + hw_],
                                 start=True, stop=False,
                                 skip_group_check=True)
                nc.tensor.matmul(psb[:, h0:h0 + hw_], self.ones1[0:1, :],
                                 self.srow[s][0:1, off + h0:off + h0 + hw_],
                                 start=False, stop=True,
                                 skip_group_check=True)
            for g in range(G):
                nc.scalar.activation(sgt[g][:, off:off + wdt],
                                     psb[:, 0:wdt], AF.Sigmoid,
                                     scale=self.sqw[g][:, s:s + 1])
        for g in range(G):
            nc.vector.tensor_scalar(out=sgt[g][:], in0=sgt[g][:],
                                    scalar1=1.0, scalar2=None, op0=ALU.add)
            nc.vector.tensor_tensor(tl[g][:], tl[g][:], sgt[g][:],
                                    op=ALU.mult)
            nc.sync.dma_start(self.y_d.ap()[s, g * 128:(g + 1) * 128, :],
                              tl[g][:])


_NC_CACHE = {}


def _get_program():
    if "nc" not in _NC_CACHE:
        _NC_CACHE["nc"] = build_program()
    return _NC_CACHE["nc"]


def _host_params(w1, b1, w2, b2, conv_w, bn_gamma, bn_beta, bn_mean, bn_var):
    w1 = np.asarray(w1, np.float32)
    w2 = np.asarray(w2, np.float32)
    b1 = np.asarray(b1, np.float32)
    b2 = np.asarray(b2, np.float32)
    conv_w = np.asarray(conv_w, np.float32)

    w1e = np.ascontiguousarray(
        w1[:, 0::2].T.reshape(4, 64, RED).transpose(1, 0, 2)
        .reshape(64, 4 * RED)).astype(np.float16)
    w1o = np.ascontiguousarray(
        w1[:, 1::2].T.reshape(4, 64, RED).transpose(1, 0, 2)
        .reshape(64, 4 * RED)).astype(np.float16)
    w2t = np.ascontiguousarray(w2.T).astype(np.float16)    # [32, 512]
    b1c = b1.reshape(RED, 1).copy()
    b2c = np.ascontiguousarray(b2.reshape(G, 128).T)       # [128, G]

    bn_scale = float(bn_gamma[0]) / np.sqrt(float(bn_var[0]) + 1e-5)
    k2 = float(bn_beta[0]) - float(bn_mean[0]) * bn_scale
    wcf = conv_w[0].astype(np.float64) * bn_scale          # [2, 7, 7]
    wcf = wcf.copy()
    wcf[0] /= C                                            # mean channel fold
    wc99 = np.concatenate([wcf.reshape(98, 1),
                           np.array([[k2]])], axis=0)      # [99, 1]
    wcrep = np.ascontiguousarray(
        np.broadcast_to(wc99, (99, 128))).astype(np.float16)

    sortscale = np.array([1.0 / HW, 1.0 / HW, 1.0, 1.0],
                         np.float32).reshape(4, 1)
    ident = np.eye(128, dtype=np.float32)
    pad0 = np.zeros(S * 2 * PW * PW, np.float16)
    onesrow = np.ones((1, HW), np.float16)
    return dict(w1e=w1e, w1o=w1o, w2t=w2t, b1c=b1c, b2c=b2c, wcrep=wcrep,
                ident=ident, sortscale=sortscale, pad0=pad0, onesrow=onesrow)


def kernel(x, w1, b1, w2, b2, conv_w, bn_gamma, bn_beta, bn_mean, bn_var):
    x = np.asarray(x, np.float32)
    params = _host_params(w1, b1, w2, b2, conv_w,
                          bn_gamma, bn_beta, bn_mean, bn_var)
    nc = _get_program()

    xr = x.reshape(B, C, HW).astype(np.float16)
    in_maps = []
    for k in range(NCORES):
        m = {"x": np.ascontiguousarray(xr[k * S:(k + 1) * S])}
        m.update(params)
        in_maps.append(m)

    res = bass_utils.run_bass_kernel_spmd(nc, in_maps,
                                          core_ids=list(range(NCORES)))
    out = np.concatenate([np.asarray(res.results[k]["y"], np.float32)
                          for k in range(NCORES)], axis=0)
    return out.reshape(B, C, H, W)


# revision 14
# speedup vs baseline: 1.9564x; 1.0115x over previous
"""ChannelGate (topk_masking) Trainium2 Bass kernel.

Strategy: pure data parallel over batch (B=32 -> 4 samples per core x 8 cores).
fp16 end-to-end: host casts x to fp16 (halves HBM reads), y is written fp16
(halves writes).  x c-tiles 0-2 stay resident in SBUF between the stats pass
and the gating pass; tile 3 is re-streamed.

Op selection is driven by measured TRN2 DVE rates: tensor_copy/tensor_scalar
~0.37 ns/elem, tensor_tensor ~0.6, reduce ~1.2, scalar_tensor_tensor ~1.3
(no fast mode) -- so everything elementwise uses TT/TS, never STT.

Two-sample groups pipeline through three phases so the top-k sort and MLP of
group 0 overlap phase 1 of group 1, and gating of group 0 overlaps the tail:
  phase 1 (per sample): ch-sum via ACT copy+accum, ch-max via TT fold tree +
           small reduce, pixel sum via 4-way accumulating PE ones-matmuls,
           pixel max via TT max tree + GPSIMD partition_all_reduce; conv
           im2col staged via a pre-zeroed DRAM pad buffer.
  phase 2 (per group): top-256 sorted: 16 x max8/match_replace on a [16, 128]
           quarter layout (fp16), exact bitonic merges on [4, 512] rows;
           tiny MLP on PE (interleave folded into host-split even/odd W1).
  phase 3 (per sample): psum = wcrep.T @ im2col + ones x sig-spatial row
           (conv, BN, bias folded host-side; bcast replicated to 128 rows);
           ACT sigmoid with per-channel scale; DVE TS (+1) and TT (*x) in
           place over the resident x tile; DMA out fp16.
"""
import numpy as np
from contextlib import ExitStack

import concourse.bass as bass
import concourse.tile as tile
from concourse import bacc, mybir, bass_isa
from concourse import bass_utils

F32 = mybir.dt.float32
F16 = mybir.dt.float16
F8 = mybir.dt.float8e4
AF = mybir.ActivationFunctionType
ALU = mybir.AluOpType
AX = mybir.AxisListType

B, C, H, W = 32, 512, 56, 56
HW = H * W            # 3136
S = 4                 # samples per core
NCORES = 8
G = 4                 # c-tiles of 128 per sample
RED = 32              # MLP hidden
PW = 62               # padded conv map width/height
CH512 = [(i * 512, min(512, HW - i * 512)) for i in range((HW + 511) // 512)]
CH1K = [(0, 1024), (1024, 1024), (2048, 1024), (3072, 64)]
NEG = -60000.0
HALF = HW // 2        # 1568


def build_program():
    nc = bacc.Bacc("TRN2", target_bir_lowering=False, debug=False,
                   num_devices=NCORES)

    x_d = nc.dram_tensor("x", [S, C, HW], F16, kind="ExternalInput")
    y_d = nc.dram_tensor("y", [S, C, HW], F16, kind="ExternalOutput")
    w1e_d = nc.dram_tensor("w1e", [64, 4 * RED], F16, kind="ExternalInput")
    w1o_d = nc.dram_tensor("w1o", [64, 4 * RED], F16, kind="ExternalInput")
    w2t_d = nc.dram_tensor("w2t", [RED, C], F16, kind="ExternalInput")
    b1_d = nc.dram_tensor("b1c", [RED, 1], F32, kind="ExternalInput")
    b2_d = nc.dram_tensor("b2c", [128, G], F32, kind="ExternalInput")
    wcr_d = nc.dram_tensor("wcrep", [99, 128], F16, kind="ExternalInput")
    id_d = nc.dram_tensor("ident", [128, 128], F32, kind="ExternalInput")
    ssc_d = nc.dram_tensor("sortscale", [4, 1], F32, kind="ExternalInput")
    pad_d = nc.dram_tensor("pad0", [S * 2 * PW * PW], F16, kind="ExternalInput")
    or_d = nc.dram_tensor("onesrow", [1, HW], F16, kind="ExternalInput")

    with tile.TileContext(nc) as tc:
        with ExitStack() as ctx:
            Kern(ctx, tc, x_d, y_d, w1e_d, w1o_d, w2t_d, b1_d, b2_d,
                 wcr_d, id_d, ssc_d, pad_d, or_d).build()
    nc.compile()
    return nc


class Kern:
    def __init__(self, ctx, tc, x_d, y_d, w1e_d, w1o_d, w2t_d, b1_d, b2_d,
                 wcr_d, id_d, ssc_d, pad_d, or_d):
        self.ctx, self.tc, self.nc = ctx, tc, tc.nc
        self.x_d, self.y_d = x_d, y_d
        self.w1e_d, self.w1o_d, self.w2t_d = w1e_d, w1o_d, w2t_d
        self.b1_d, self.b2_d, self.wcr_d = b1_d, b2_d, wcr_d
        self.id_d, self.ssc_d, self.pad_d, self.or_d = (id_d, ssc_d, pad_d,
                                                        or_d)

    def build(self):
        ctx, tc, nc = self.ctx, self.tc, self.nc
        cpool = ctx.enter_context(tc.tile_pool(name="consts", bufs=1))
        self.big = ctx.enter_context(tc.tile_pool(name="big", bufs=5))
        rowp = ctx.enter_context(tc.tile_pool(name="rows", bufs=1))
        self.ps_pix = ctx.enter_context(
            tc.tile_pool(name="ps_pix", bufs=2, space="PSUM"))
        self.ps_sm = ctx.enter_context(
            tc.tile_pool(name="ps_sm", bufs=2, space="PSUM"))
        self.ps_bc = ctx.enter_context(
            tc.tile_pool(name="ps_bc", bufs=2, space="PSUM"))

        # resident x tiles + first sample loads go out before the consts
        self.xt = [[cpool.tile([128, HW], F16, tag=f"x{s}{g}", name=f"x{s}{g}")
                    for g in range(G)] for s in range(S)]
        self.load_x(0)

        ident = cpool.tile([128, 128], F32)
        nc.sync.dma_start(ident[:], self.id_d.ap())
        self.ident = ident
        self.ones1 = cpool.tile([1, 128], F16)
        nc.vector.memset(self.ones1[:], 1.0)
        self.onescol = cpool.tile([128, 1], F16)
        nc.vector.memset(self.onescol[:], 1.0)
        self.w1e = cpool.tile([64, 4 * RED], F16)
        nc.sync.dma_start(self.w1e[:], self.w1e_d.ap())
        self.w1o = cpool.tile([64, 4 * RED], F16)
        nc.sync.dma_start(self.w1o[:], self.w1o_d.ap())
        self.w2t = cpool.tile([RED, C], F16)
        nc.sync.dma_start(self.w2t[:], self.w2t_d.ap())
        self.b1 = cpool.tile([RED, 1], F32)
        nc.sync.dma_start(self.b1[:], self.b1_d.ap())
        self.b2 = cpool.tile([128, G], F32)
        nc.sync.dma_start(self.b2[:], self.b2_d.ap())
        self.wcrep = cpool.tile([99, 128], F16)
        nc.sync.dma_start(self.wcrep[:], self.wcr_d.ap())
        self.ssc = cpool.tile([4, 1], F32)
        nc.sync.dma_start(self.ssc[:], self.ssc_d.ap())
        self.ident16 = cpool.tile([8, 8], F16)
        nc.vector.tensor_copy(self.ident16[:], ident[0:8, 0:8])

        self.sc8 = [cpool.tile([128, 8], F32, tag=f"sc8{g}", name=f"sc8{g}")
                    for g in range(G)]
        self.srow = [rowp.tile([1, HW], F16, tag=f"srow{s}", name=f"srow{s}")
                     for s in range(S)]
        self.imts = [cpool.tile([99, HW], F16, tag=f"imt{s}", name=f"imt{s}")
                     for s in range(S)]
        for s in range(S):
            nc.gpsimd.dma_start(self.imts[s][98:99, :], self.or_d.ap())
        self.fold = cpool.tile([128, HALF], F16)
        self.scrap = cpool.tile([128, HW], F8)

        # per-group sort tiles
        self.srtF = [cpool.tile([4, C], F16, tag=f"srtF{k}", name=f"srtF{k}")
                     for k in range(2)]
        self.srt16 = [cpool.tile([16, 128], F16, tag=f"sr16{k}",
                                 name=f"sr16{k}") for k in range(2)]
        self.sorted16 = [cpool.tile([16, 128], F16, tag=f"so16{k}",
                                    name=f"so16{k}") for k in range(2)]
        self.q512 = [cpool.tile([4, C], F16, tag=f"q512{k}", name=f"q512{k}")
                     for k in range(2)]
        self.mg1 = [cpool.tile([4, C], F16, tag=f"mg1{k}", name=f"mg1{k}")
                    for k in range(2)]
        self.mg2 = [cpool.tile([4, C], F16, tag=f"mg2{k}", name=f"mg2{k}")
                    for k in range(2)]
        self.tq = [cpool.tile([64, 4], F16, tag=f"tq{q}", name=f"tq{q}")
                   for q in range(4)]
        self.h_sb = [cpool.tile([RED, 2], F16, tag=f"hsb{k}", name=f"hsb{k}")
                     for k in range(2)]
        self.sqw = [cpool.tile([128, S], F32, tag=f"sqw{g}", name=f"sqw{g}")
                    for g in range(G)]
        self.prod = [cpool.tile([128, 2], F32, tag=f"prod{g}",
                                name=f"prod{g}") for g in range(G)]
        self.sigp = [cpool.tile([128, 2], F32, tag=f"sigp{g}",
                                name=f"sigp{g}") for g in range(G)]

        # ---- pipeline ----
        self.phase1(0)
        self.load_x(1)
        self.phase1(1)
        self.load_x(2)
        self.sort_mlp(0)
        self.phase1(2)
        self.load_x(3)
        self.phase3(0)
        self.phase1(3)
        self.phase3(1)
        self.sort_mlp(1)
        self.phase3(2)
        self.phase3(3)

    def load_x(self, s):
        nc = self.nc
        for g in range(G):
            nc.sync.dma_start(self.xt[s][g][:],
                              self.x_d.ap()[s, g * 128:(g + 1) * 128, :])

    def phase1(self, s):
        nc = self.nc
        ts = self.xt[s]
        sc8, srow, fold = self.sc8, self.srow[s], self.fold

        Bt = self.big.tile([128, HW], F16, tag="big", name=f"Bt{s}")
        Ct = self.big.tile([128, HW], F16, tag="big", name=f"Ct{s}")

        cs = 4 * (s // 2) + (s % 2)      # per-group col block: s0,s1,m0,m1
        cm = cs + 2
        for g in range(G):
            nc.scalar.activation(self.scrap[:], ts[g][:], AF.Copy,
                                 accum_out=sc8[g][:, cs:cs + 1])
            nc.vector.tensor_tensor(fold[:], ts[g][:, 0:HALF],
                                    ts[g][:, HALF:HW], op=ALU.max)
            nc.vector.tensor_tensor(fold[:, 0:784], fold[:, 0:784],
                                    fold[:, 784:1568], op=ALU.max)
            nc.vector.reduce_max(sc8[g][:, cm:cm + 1], fold[:, 0:784],
                                 axis=AX.X)

        for (off, wdt) in CH512:
            ps = self.ps_pix.tile([1, 512], F32, tag="pix")
            for g in range(G):
                nc.tensor.matmul(ps[0:1, 0:wdt], self.onescol[:, 0:1],
                                 ts[g][:, off:off + wdt],
                                 start=(g == 0), stop=(g == 3))
            nc.scalar.copy(srow[0:1, off:off + wdt], ps[0:1, 0:wdt])

        nc.vector.tensor_tensor(Ct[:], ts[0][:], ts[1][:], op=ALU.max)
        nc.vector.tensor_tensor(Bt[:], ts[2][:], ts[3][:], op=ALU.max)
        nc.vector.tensor_tensor(Ct[:], Ct[:], Bt[:], op=ALU.max)
        nc.gpsimd.partition_all_reduce(Bt[:], Ct[:], channels=128,
                                       reduce_op=bass_isa.ReduceOp.max)

        base0 = ((s * 2 + 0) * PW + 3) * PW + 3
        base1 = ((s * 2 + 1) * PW + 3) * PW + 3
        nc.sync.dma_start(
            bass.AP(self.pad_d, base0, [[PW, H], [1, W]]),
            srow[0:1, 0:HW].rearrange("p (h w) -> p h w", h=H))
        nc.gpsimd.dma_start(
            bass.AP(self.pad_d, base1, [[PW, H], [1, W]]),
            Bt[0:1, 0:HW].rearrange("p (h w) -> p h w", h=H))

        nc.vector.tensor_tensor(srow[0:1, 0:HW], srow[0:1, 0:HW],
                                Bt[0:1, 0:HW], op=ALU.mult)
        nc.scalar.activation(srow[0:1, 0:HW], srow[0:1, 0:HW],
                             AF.Sigmoid, scale=1.0 / C)

        # im2col loads: ci 0 on HWDGE (sync), ci 1 on SWDGE (gpsimd)
        for ci in range(2):
            eng = nc.sync if ci == 0 else nc.gpsimd
            for kh in range(7):
                base = ((s * 2 + ci) * PW + kh) * PW
                src = bass.AP(self.pad_d, base, [[1, 7], [PW, H], [1, W]])
                p0 = ci * 49 + kh * 7
                eng.dma_start(self.imts[s][p0:p0 + 7, :], src)

    def sort_mlp(self, k):
        nc = self.nc
        s0 = 2 * k
        srtF, srt16, sorted16 = self.srtF[k], self.srt16[k], self.sorted16[k]
        q512, mg1, mg2 = self.q512[k], self.mg1[k], self.mg2[k]

        # stats -> [4, 512] rows (r = pool*2 + si), sums scaled by 1/HW
        for g in range(G):
            pst = self.ps_sm.tile([4, 128], F32, tag="pst")
            nc.tensor.transpose(pst[:], self.sc8[g][:, 4 * k:4 * k + 4],
                                self.ident[:])
            nc.scalar.activation(srtF[:, g * 128:(g + 1) * 128], pst[:],
                                 AF.Copy, scale=self.ssc[:])
        for g in range(G):
            nc.sync.dma_start(srt16[4 * g:4 * g + 4, :],
                              srtF[0:4, g * 128:(g + 1) * 128])
        for it in range(16):
            m8 = sorted16[:, 8 * it:8 * it + 8]
            nc.vector.max(out=m8, in_=srt16[:])
            nc.vector.match_replace(out=srt16[:], in_to_replace=m8,
                                    in_values=srt16[:], imm_value=NEG)
        for g in range(G):
            nc.sync.dma_start(q512[0:4, g * 128:(g + 1) * 128],
                              sorted16[4 * g:4 * g + 4, :])
        # merge quarters pairwise (both pairs at once)
        nc.vector.tensor_tensor(mg1[:, 0:128], q512[:, 0:128],
                                q512[:, 255:127:-1], op=ALU.max)
        nc.vector.tensor_tensor(mg1[:, 128:256], q512[:, 0:128],
                                q512[:, 255:127:-1], op=ALU.min)
        nc.vector.tensor_tensor(mg1[:, 256:384], q512[:, 256:384],
                                q512[:, 511:383:-1], op=ALU.max)
        nc.vector.tensor_tensor(mg1[:, 384:512], q512[:, 256:384],
                                q512[:, 511:383:-1], op=ALU.min)
        cur, nxt = mg1, mg2
        for d in (64, 32, 16, 8, 4, 2, 1):
            cv = cur[:].rearrange("p (x two d) -> p x two d", two=2, d=d)
            nv = nxt[:].rearrange("p (x two d) -> p x two d", two=2, d=d)
            nc.vector.tensor_tensor(nv[:, :, 0, :], cv[:, :, 0, :],
                                    cv[:, :, 1, :], op=ALU.max)
            nc.vector.tensor_tensor(nv[:, :, 1, :], cv[:, :, 0, :],
                                    cv[:, :, 1, :], op=ALU.min)
            cur, nxt = nxt, cur
        nc.vector.tensor_tensor(nxt[:, 0:256], cur[:, 0:256],
                                cur[:, 511:255:-1], op=ALU.max)
        cur, nxt = nxt, cur
        for d in (128, 64, 32, 16, 8, 4, 2, 1):
            cv = cur[:, 0:256].rearrange("p (x two d) -> p x two d",
                                         two=2, d=d)
            nv = nxt[:, 0:256].rearrange("p (x two d) -> p x two d",
                                         two=2, d=d)
            nc.vector.tensor_tensor(nv[:, :, 0, :], cv[:, :, 0, :],
                                    cv[:, :, 1, :], op=ALU.max)
            nc.vector.tensor_tensor(nv[:, :, 1, :], cv[:, :, 0, :],
                                    cv[:, :, 1, :], op=ALU.min)
            cur, nxt = nxt, cur
        # cur[:, 0:256] = top-256 sorted desc; rows 0-1 = t1, 2-3 = t2
        for q in range(4):
            pstq = self.ps_sm.tile([64, 4], F16, tag="pst")
            nc.tensor.transpose(pstq[:], cur[0:4, 64 * q:64 * q + 64],
                                self.ident16[0:4, 0:4])
            nc.scalar.copy(self.tq[q][:], pstq[:])
        psh = self.ps_sm.tile([RED, 2], F32, tag="pst")
        for q in range(4):
            c0 = q * RED
            nc.tensor.matmul(psh[:], self.w1e[:, c0:c0 + RED],
                             self.tq[q][:, 0:2], start=(q == 0), stop=False)
            nc.tensor.matmul(psh[:], self.w1o[:, c0:c0 + RED],
                             self.tq[q][:, 2:4], start=False, stop=(q == 3))
        nc.scalar.activation(self.h_sb[k][:], psh[:], AF.Relu, bias=self.b1[:])
        for g in range(G):
            psm = self.ps_sm.tile([128, 2], F32, tag="pst")
            nc.tensor.matmul(psm[:], self.w2t[:, g * 128:(g + 1) * 128],
                             self.h_sb[k][:], start=True, stop=True)
            nc.vector.tensor_tensor(self.prod[g][:],
                                    self.sc8[g][:, 4 * k:4 * k + 2],
                                    self.sc8[g][:, 4 * k + 2:4 * k + 4],
                                    op=ALU.mult)
            nc.scalar.activation(self.sigp[g][:], self.prod[g][:], AF.Sigmoid,
                                 scale=1.0 / HW)
            nc.vector.tensor_tensor(self.sigp[g][:], self.sigp[g][:], psm[:],
                                    op=ALU.add)
            nc.scalar.activation(self.sqw[g][:, s0:s0 + 2], self.sigp[g][:],
                                 AF.Relu, bias=self.b2[:, g:g + 1])

    def phase3(self, s):
        nc = self.nc
        tl = self.xt[s]
        sgt = [self.big.tile([128, HW], F16, tag="big", name=f"sgt{s}{g}")
               for g in range(G)]
        for (off, wdt) in CH1K:
            psb = self.ps_bc.tile([128, 1024], F32, tag="bc")
            for h0 in range(0, wdt, 512):
                hw_ = min(512, wdt - h0)
                nc.tensor.matmul(psb[:, h0:h0 + hw_], self.wcrep[:],
                                 self.imts[s][:, off + h0:off + h0 + hw_],
                                 start=True, stop=False,
                                 skip_group_check=True)
                nc.tensor.matmul(psb[:, h0:h0 + hw_], self.ones1[0:1, :],
                                 self.srow[s][0:1, off + h0:off + h0 + hw_],
                                 start=False, stop=True,
                                 skip_group_check=True)
            for g in range(G):
                nc.scalar.activation(sgt[g][:, off:off + wdt],
                                     psb[:, 0:wdt], AF.Sigmoid,
                                     scale=self.sqw[g][:, s:s + 1])
        for g in range(G):
            nc.vector.tensor_scalar(out=sgt[g][:], in0=sgt[g][:],
                                    scalar1=1.0, scalar2=None, op0=ALU.add)
            nc.vector.tensor_tensor(tl[g][:], tl[g][:], sgt[g][:],
                                    op=ALU.mult)
            nc.sync.dma_start(self.y_d.ap()[s, g * 128:(g + 1) * 128, :],
                              tl[g][:])


_NC_CACHE = {}


def _get_program():
    if "nc" not in _NC_CACHE:
        _NC_CACHE["nc"] = build_program()
    return _NC_CACHE["nc"]


def _host_params(w1, b1, w2, b2, conv_w, bn_gamma, bn_beta, bn_mean, bn_var):
    w1 = np.asarray(w1, np.float32)
    w2 = np.asarray(w2, np.float32)
    b1 = np.asarray(b1, np.float32)
    b2 = np.asarray(b2, np.float32)
    conv_w = np.asarray(conv_w, np.float32)

    w1e = np.ascontiguousarray(
        w1[:, 0::2].T.reshape(4, 64, RED).transpose(1, 0, 2)
        .reshape(64, 4 * RED)).astype(np.float16)
    w1o = np.ascontiguousarray(
        w1[:, 1::2].T.reshape(4, 64, RED).transpose(1, 0, 2)
        .reshape(64, 4 * RED)).astype(np.float16)
    w2t = np.ascontiguousarray(w2.T).astype(np.float16)    # [32, 512]
    b1c = b1.reshape(RED, 1).copy()
    b2c = np.ascontiguousarray(b2.reshape(G, 128).T)       # [128, G]

    bn_scale = float(bn_gamma[0]) / np.sqrt(float(bn_var[0]) + 1e-5)
    k2 = float(bn_beta[0]) - float(bn_mean[0]) * bn_scale
    wcf = conv_w[0].astype(np.float64) * bn_scale          # [2, 7, 7]
    wcf = wcf.copy()
    wcf[0] /= C                                            # mean channel fold
    wc99 = np.concatenate([wcf.reshape(98, 1),
                           np.array([[k2]])], axis=0)      # [99, 1]
    wcrep = np.ascontiguousarray(
        np.broadcast_to(wc99, (99, 128))).astype(np.float16)

    sortscale = np.array([1.0 / HW, 1.0 / HW, 1.0, 1.0],
                         np.float32).reshape(4, 1)
    ident = np.eye(128, dtype=np.float32)
    pad0 = np.zeros(S * 2 * PW * PW, np.float16)
    onesrow = np.ones((1, HW), np.float16)
    return dict(w1e=w1e, w1o=w1o, w2t=w2t, b1c=b1c, b2c=b2c, wcrep=wcrep,
                ident=ident, sortscale=sortscale, pad0=pad0, onesrow=onesrow)


def kernel(x, w1, b1, w2, b2, conv_w, bn_gamma, bn_beta, bn_mean, bn_var):
    x = np.asarray(x, np.float32)
    params = _host_params(w1, b1, w2, b2, conv_w,
                          bn_gamma, bn_beta, bn_mean, bn_var)
    nc = _get_program()

    xr = x.reshape(B, C, HW).astype(np.float16)
    in_maps = []
    for k in range(NCORES):
        m = {"x": np.ascontiguousarray(xr[k * S:(k + 1) * S])}
        m.update(params)
        in_maps.append(m)

    res = bass_utils.run_bass_kernel_spmd(nc, in_maps,
                                          core_ids=list(range(NCORES)))
    out = np.concatenate([np.asarray(res.results[k]["y"], np.float32)
                          for k in range(NCORES)], axis=0)
    return out.reshape(B, C, H, W)
